# revision 1
# baseline (speedup 1.0000x reference)
"""Self-contained Trainium2 Bass kernel: pre-LN multi-head attention block.

Computes, for x [B=8, S=1024, D=1024] (fp32) and packed attention weights:
    out = x + out_proj(MHA(LayerNorm(x)))
matching torch nn.MultiheadAttention's explicit (non-flash) path with 16 heads.

Sharding: data-parallel over batch — core i handles batch element i; no
collectives, outputs are concatenated on the host.

Per-core layout strategy (transposed activations, d on partitions):
  - host supplies xT = x[i].T (bf16) so LN + projections run with the
    contraction dim (d) on SBUF partitions; LN stats (sums over d) are
    matmuls against an all-ones [128,128] stationary, which lands the sums
    partition-replicated in PSUM (broadcast for free); the whole LN
    pipeline is chunked over 512-column halves so stats DMA, the
    mean/rstd chain, normalize, and the V projection overlap.
  - QKV projection emits Q^T,K^T (head dim on partitions) and V in natural
    layout [t, e'], each head's V augmented with a ones column so the
    PV matmul also produces the softmax denominator (PSUM row 64); the V
    projection runs d-tile-outer across 4 concurrent PSUM groups to keep
    pace with the LN normalize.
  - scores^T[t, s] = K^T.T @ Q^T per head (K=64 contraction, two heads
    row-packed via tile_position); softmax exp runs on the scalar engine
    straight out of PSUM with the 1/sqrt(dh) scale folded in; no max
    subtraction (scores are O(few) by construction).
  - ctx^T normalization by 1/denominator is deferred to just before the
    out-projection (denominator reciprocals broadcast via a DRAM
    round-trip); the out-projection lands in natural [s, e] layout for
    the residual add. PSUM tags are split per logical stream (qk-proj /
    scores / PV / out-proj) so phases overlap without bank conflicts.
"""

import numpy as np
import ml_dtypes

P = 128
D = 1024
H = 16
DH = 64
E = 3 * D
B = 8
S = 1024
LN_EPS = 1e-5
N_CORES = 8

_ND = D // P   # d tiles (8)
_NC = 512      # matmul moving chunk

LAST_RESULTS = None
_NC_CACHE = {}


def _emit(tc, aps, S_=S):
    from concourse import mybir

    nc = tc.nc
    f32 = mybir.dt.float32
    bf16 = mybir.dt.bfloat16
    FT = mybir.ActivationFunctionType
    OP = mybir.AluOpType

    ns = S_ // P
    ncs = max(1, S_ // _NC)
    NCK = min(_NC, S_)
    nqk = 2 * D // P  # q+k e-tiles (16)

    xT, xnat, winT, woutT, gammat, betat, binqk, binv, bout, out = (
        aps["xt"], aps["xnat"], aps["wint"], aps["woutt"], aps["gammat"],
        aps["betat"], aps["binqk"], aps["binv"], aps["bout"], aps["out"],
    )
    winT_r = winT.rearrange("(a p) e -> p a e", p=P)

    with tc.tile_pool(name="consts", bufs=1) as consts, \
         tc.tile_pool(name="acts", bufs=1) as acts, \
         tc.tile_pool(name="winv", bufs=1) as wvpool, \
         tc.tile_pool(name="dscratch", bufs=1, space="DRAM") as dscratch:

        # V-column weights (DMA issued after the x chunks, below)
        winv_sb = wvpool.tile([P, _ND, D], bf16, tag="w")

        # ---------- constants (DMAs issued after the x chunks, below) ----------
        cvec = consts.tile([P, _ND + _ND + nqk + 1], f32)
        gamma_sb = cvec[:, 0:_ND]
        beta_sb = cvec[:, _ND:2 * _ND]
        binqk_sb = cvec[:, 2 * _ND:2 * _ND + nqk]
        eps_sb = cvec[:, 2 * _ND + nqk:2 * _ND + nqk + 1]
        nc.vector.memset(eps_sb, LN_EPS)
        ones_mat = consts.tile([P, P], bf16)
        nc.vector.memset(ones_mat, 1.0)
        binv_bc = consts.tile([P, D], f32)
        bout_bc = consts.tile([P, D], f32)

        # ---------- persistent activations ----------
        xnT_sb = acts.tile([P, _ND, S_], bf16)      # normalized x, transposed
        qkT_sb = acts.tile([P, nqk, S_], bf16)      # q (tiles 0..7), k (8..15)
        v_sb = acts.tile([P, ns, H, DH + 1], bf16)  # v natural + ones column
        ctx_sb = acts.tile([P, _ND, S_], bf16)      # ctx^T, normalized in place
        # softmax 1/denominator: head h at partition 32*(h//4), slot h%4
        # (SBUF engine APs may only start at partitions 0/32/64/96)
        den_sb = acts.tile([P, 4, S_], bf16)
        xnat01 = acts.tile([P, 2, D], f32)          # residual prefetch (st 0,1)
        rd_dram = dscratch.tile([H, S_], bf16)

        # ================= Phase 1: LayerNorm =================
        with tc.tile_pool(name="lnsb", bufs=1) as lnsb, \
             tc.tile_pool(name="lnrow", bufs=1) as lnrow, \
             tc.tile_pool(name="lntmp", bufs=2) as lntmp, \
             tc.tile_pool(name="lnps", bufs=1, space="PSUM") as lnps:
            xT_sb = lnsb.tile([P, _ND, S_], bf16)
            # all-ones [P,P] stationary: stats sums land partition-replicated
            # in PSUM (no broadcast step); the whole LN pipeline is chunked
            # over 512-column halves so half 1's stats overlap half 0's
            # normalize and the V projection starts as early as possible
            sx_ps = lnps.tile([P, S_], f32, tag="sx")
            sx2_ps = lnps.tile([P, S_], f32, tag="sx2")
            xT_r = xT.rearrange("(a p) s -> p a s", p=P)
            for c in range(ncs):
                sl = slice(c * NCK, (c + 1) * NCK)
                for j in range(_ND):
                    nc.sync.dma_start(out=xT_sb[:, j, sl], in_=xT_r[:, j, sl])
                for j in range(_ND):
                    sq = lntmp.tile([P, NCK], bf16, tag="sq", bufs=4)
                    with nc.allow_low_precision(reason="x^2 for LN stats in bf16"):
                        nc.vector.tensor_tensor(out=sq, in0=xT_sb[:, j, sl],
                                                in1=xT_sb[:, j, sl], op=OP.mult)
                    nc.tensor.matmul(sx_ps[:, sl], lhsT=ones_mat, rhs=xT_sb[:, j, sl],
                                     start=(j == 0), stop=(j == _ND - 1))
                    nc.tensor.matmul(sx2_ps[:, sl], lhsT=ones_mat, rhs=sq,
                                     start=(j == 0), stop=(j == _ND - 1))
                if c == 0:
                    nc.sync.dma_start(out=winv_sb, in_=winT_r[:, :, 2 * D:])
                    nc.sync.dma_start(out=cvec[:, 0:_ND], in_=gammat)
                    nc.sync.dma_start(out=cvec[:, _ND:2 * _ND], in_=betat)
                    nc.sync.dma_start(out=cvec[:, 2 * _ND:2 * _ND + nqk], in_=binqk)
                    nc.gpsimd.dma_start(out=binv_bc,
                                        in_=binv[None, :].to_broadcast((P, D)))
                    nc.gpsimd.dma_start(out=bout_bc,
                                        in_=bout[None, :].to_broadcast((P, D)))

                mu_bc = lnrow.tile([P, NCK], f32, tag="mu", bufs=2)
                nc.vector.tensor_scalar_mul(mu_bc, sx_ps[:, sl], 1.0 / D)
                var_bc = lnrow.tile([P, NCK], f32, tag="var", bufs=2)
                nc.vector.tensor_scalar_mul(var_bc, sx2_ps[:, sl], 1.0 / D)
                musq = lnrow.tile([P, NCK], f32, tag="musq", bufs=2)
                nc.vector.tensor_tensor(out=musq, in0=mu_bc, in1=mu_bc, op=OP.mult)
                nc.vector.tensor_tensor(out=var_bc, in0=var_bc, in1=musq,
                                        op=OP.subtract)
                std_bc = musq
                nc.scalar.activation(out=std_bc, in_=var_bc, func=FT.Sqrt, bias=eps_sb)
                b_bc = var_bc
                nc.vector.reciprocal(out=b_bc, in_=std_bc)                   # B
                mub_bc = std_bc
                nc.vector.tensor_tensor(out=mub_bc, in0=mu_bc, in1=b_bc, op=OP.mult)

                for j in range(_ND):
                    t = lntmp.tile([P, NCK], f32, tag="nrm", bufs=4)
                    nc.vector.tensor_tensor(out=t, in0=xT_sb[:, j, sl], in1=b_bc,
                                            op=OP.mult)
                    nc.vector.tensor_tensor(out=t, in0=t, in1=mub_bc, op=OP.subtract)
                    nc.scalar.activation(out=xnT_sb[:, j, sl], in_=t, func=FT.Identity,
                                         bias=beta_sb[:, j:j + 1],
                                         scale=gamma_sb[:, j:j + 1])

        # ============ Phases 2-4: projections + attention ============
        with tc.tile_pool(name="winqk", bufs=1) as wqpool, \
             tc.tile_pool(name="expp", bufs=2) as expp:

            winqk_sb = wqpool.tile([P, _ND, 2 * D], bf16)
            nc.sync.dma_start(out=winqk_sb, in_=winT_r[:, :, 0:2 * D])
            for st in range(min(2, ns)):
                nc.sync.dma_start(out=xnat01[:, st], in_=xnat[st * P:(st + 1) * P, :])

            # ones column of the augmented V blocks
            nc.vector.memset(v_sb[:, :, :, DH:DH + 1], 1.0)

            # ---- V projection (natural layout [t, e']) ----
            # j-outer with 4 concurrent PSUM groups so the PE keeps pace with
            # the LN normalize producing one xnT d-tile every ~1.7us
            with tc.tile_pool(name="vps", bufs=1, space="PSUM") as vps:
                nhalf = max(1, ns // 4)
                for half in range(nhalf):
                    nst = min(4, ns)
                    tiles = [vps.tile([P, D], f32, tag="vp", bufs=4,
                                      name=f"vp{half}_{k}") for k in range(nst)]
                    for j in range(_ND):
                        for k in range(nst):
                            st = half * 4 + k
                            for c in range(D // _NC):
                                sl = slice(c * _NC, (c + 1) * _NC)
                                nc.tensor.matmul(
                                    tiles[k][:, sl],
                                    lhsT=xnT_sb[:, j, st * P:(st + 1) * P],
                                    rhs=winv_sb[:, j, sl],
                                    start=(j == 0), stop=(j == _ND - 1))
                    for k in range(nst):
                        st = half * 4 + k
                        nc.vector.tensor_tensor(
                            out=v_sb[:, st, :, 0:DH],
                            in0=tiles[k].rearrange("p (h d) -> p h d", d=DH),
                            in1=binv_bc.rearrange("p (h d) -> p h d", d=DH),
                            op=OP.add)

                # pair-0 Q/K projections ride in this pool's slots so the
                # attention loop starts without waiting on the pool swap
                if S_ >= D:
                    for et in (0, 8):
                        qp = vps.tile([P, D], f32, tag="vp", bufs=4,
                                      name=f"qp{et}")
                        for c in range(ncs):
                            sl = slice(c * NCK, (c + 1) * NCK)
                            for j in range(_ND):
                                nc.tensor.matmul(qp[:, sl],
                                                 lhsT=winqk_sb[:, j, et * P:(et + 1) * P],
                                                 rhs=xnT_sb[:, j, sl],
                                                 start=(j == 0), stop=(j == _ND - 1))
                        nc.vector.tensor_scalar_add(qkT_sb[:, et], qp,
                                                    binqk_sb[:, et:et + 1])

            # attention-phase PSUM pool (banks freed by the V pool above)
            with tc.tile_pool(name="mps", bufs=1, space="PSUM") as mps:

                # out-proj weights reuse the winv slot (V projection is done with it)
                woutT_sb = wvpool.tile([P, _ND, D], bf16, tag="w")
                nc.sync.dma_start(out=woutT_sb, in_=woutT.rearrange("(a p) e -> p a e", p=P))

                # ---- per head-pair: Q/K projection, scores^T, exp, PV ----
                for hp in range(H // 2):
                    for et in (() if (hp == 0 and S_ >= D) else (hp, 8 + hp)):
                        ps = mps.tile([P, S_], f32, tag="qs", bufs=1)
                        for c in range(ncs):
                            sl = slice(c * NCK, (c + 1) * NCK)
                            for j in range(_ND):
                                nc.tensor.matmul(ps[:, sl],
                                                 lhsT=winqk_sb[:, j, et * P:(et + 1) * P],
                                                 rhs=xnT_sb[:, j, sl],
                                                 start=(j == 0), stop=(j == _ND - 1))
                        nc.vector.tensor_scalar_add(qkT_sb[:, et], ps, binqk_sb[:, et:et + 1])

                    ex = [expp.tile([P, ns, S_], bf16, tag="exp", name=f"ex{hp}_{i}")
                          for i in range(2)]
                    for tt in range(ns):
                        pss = [mps.tile([P, S_], f32, tag="sc", bufs=2,
                                        name=f"sc{hp}_{tt}_{i}") for i in range(2)]
                        for idx in range(2):
                            base = idx * DH
                            for c in range(ncs):
                                sl = slice(c * NCK, (c + 1) * NCK)
                                nc.tensor.matmul(
                                    pss[idx][:, sl],
                                    lhsT=qkT_sb[base:base + DH, 8 + hp, tt * P:(tt + 1) * P],
                                    rhs=qkT_sb[base:base + DH, hp, sl],
                                    start=True, stop=True, tile_position=(base, 0))
                        for idx in range(2):
                            nc.scalar.activation(out=ex[idx][:, tt], in_=pss[idx],
                                                 func=FT.Exp, scale=0.125)

                    # PV with ones-augmented V: rows 0..63 ctx^T, row 64 denominator
                    for idx in range(2):
                        h = 2 * hp + idx
                        dp = 32 * (h // 4)
                        for c in range(ncs):
                            sl = slice(c * NCK, (c + 1) * NCK)
                            pv = mps.tile([DH + 1, NCK], f32, tag="pv", bufs=2)
                            for tt in range(ns):
                                nc.tensor.matmul(pv, lhsT=v_sb[:, tt, h, :],
                                                 rhs=ex[idx][:, tt, sl],
                                                 start=(tt == 0), stop=(tt == ns - 1))
                            # alternate engines per head so the PV slot frees
                            # from both sides in parallel
                            cp = nc.vector.tensor_copy if idx == 0 else \
                                (lambda out, in_: nc.scalar.copy(out=out, in_=in_))
                            cp(out=ctx_sb[idx * DH:(idx + 1) * DH, hp, sl],
                               in_=pv[0:DH, :])
                            cp(out=den_sb[dp:dp + 1, h % 4, sl],
                               in_=pv[DH:DH + 1, :])
                        # reciprocal + ship to DRAM for the later broadcast
                        with nc.allow_low_precision(reason="softmax denom in bf16"):
                            nc.vector.reciprocal(out=den_sb[dp:dp + 1, h % 4, :],
                                                 in_=den_sb[dp:dp + 1, h % 4, :])
                        nc.sync.dma_start(out=rd_dram[h:h + 1, :],
                                          in_=den_sb[dp:dp + 1, h % 4, :])

                    # normalize this pair's ctx^T now so the out-projection
                    # phase starts with everything ready (no serial tail)
                    rdbc = expp.tile([P, S_], bf16, tag="rdbc", bufs=1,
                                     name=f"rdbc{hp}")
                    nc.sync.dma_start(
                        out=rdbc[0:DH, :],
                        in_=rd_dram[2 * hp][None, :].to_broadcast((DH, S_)))
                    nc.sync.dma_start(
                        out=rdbc[DH:P, :],
                        in_=rd_dram[2 * hp + 1][None, :].to_broadcast((DH, S_)))
                    nc.vector.tensor_tensor(out=ctx_sb[:, hp], in0=ctx_sb[:, hp],
                                            in1=rdbc, op=OP.mult)

        # ============ Phase 5: normalize + out-projection ============
        with tc.tile_pool(name="p5", bufs=2) as p5, \
             tc.tile_pool(name="p5ps", bufs=1, space="PSUM") as p5ps:
            for st in range(ns):
                if st < 2 and ns > 2:
                    xn_t = xnat01[:, st]
                else:
                    xn_t = p5.tile([P, D], f32, tag="xnat", bufs=4)
                    nc.sync.dma_start(out=xn_t, in_=xnat[st * P:(st + 1) * P, :])
                nc.vector.tensor_tensor(out=xn_t, in0=xn_t, in1=bout_bc, op=OP.add)
                ot = p5.tile([P, D], f32, tag="out")
                for c in range(D // _NC):
                    sl = slice(c * _NC, (c + 1) * _NC)
                    po = p5ps.tile([P, _NC], f32, tag="po", bufs=6)
                    for j in range(_ND):
                        nc.tensor.matmul(po, lhsT=ctx_sb[:, j, st * P:(st + 1) * P],
                                         rhs=woutT_sb[:, j, sl],
                                         start=(j == 0), stop=(j == _ND - 1))
                    nc.vector.tensor_tensor(out=ot[:, sl], in0=po, in1=xn_t[:, sl], op=OP.add)
                    nc.sync.dma_start(out=out[st * P:(st + 1) * P, sl], in_=ot[:, sl])


def build_nc(S_=S):
    import concourse.bacc as bacc
    import concourse.tile as tile
    from concourse import mybir

    f32 = mybir.dt.float32
    bf16 = mybir.dt.bfloat16

    nc = bacc.Bacc("TRN2", target_bir_lowering=False, debug=False)
    aps = {
        "xt": nc.dram_tensor("xt", [D, S_], bf16, kind="ExternalInput").ap(),
        "xnat": nc.dram_tensor("xnat", [S_, D], f32, kind="ExternalInput").ap(),
        "wint": nc.dram_tensor("wint", [D, E], bf16, kind="ExternalInput").ap(),
        "woutt": nc.dram_tensor("woutt", [D, D], bf16, kind="ExternalInput").ap(),
        "gammat": nc.dram_tensor("gammat", [P, _ND], f32, kind="ExternalInput").ap(),
        "betat": nc.dram_tensor("betat", [P, _ND], f32, kind="ExternalInput").ap(),
        "binqk": nc.dram_tensor("binqk", [P, 2 * D // P], f32, kind="ExternalInput").ap(),
        "binv": nc.dram_tensor("binv", [D], f32, kind="ExternalInput").ap(),
        "bout": nc.dram_tensor("bout", [D], f32, kind="ExternalInput").ap(),
        "out": nc.dram_tensor("out", [S_, D], f32, kind="ExternalOutput").ap(),
    }
    with tile.TileContext(nc) as tc:
        _emit(tc, aps, S_)
    nc.compile()
    return nc


def prep_inputs(x, ln_gamma, ln_beta, in_proj_w, in_proj_b, out_proj_w, out_proj_b,
                S_=S, n_cores=N_CORES):
    bf = ml_dtypes.bfloat16
    f32c = lambda a: np.ascontiguousarray(a, dtype=np.float32)
    shared = {
        "wint": np.ascontiguousarray(np.asarray(in_proj_w, np.float32).T).astype(bf),
        "woutt": np.ascontiguousarray(np.asarray(out_proj_w, np.float32).T).astype(bf),
        "gammat": f32c(np.asarray(ln_gamma, np.float32).reshape(_ND, P).T),
        "betat": f32c(np.asarray(ln_beta, np.float32).reshape(_ND, P).T),
        "binqk": f32c(np.asarray(in_proj_b, np.float32)[:2 * D].reshape(2 * D // P, P).T),
        "binv": f32c(np.asarray(in_proj_b, np.float32)[2 * D:]),
        "bout": f32c(np.asarray(out_proj_b, np.float32)),
    }
    in_maps = []
    for i in range(n_cores):
        xi = np.asarray(x[i], np.float32)[:S_]
        m = dict(shared)
        m["xt"] = np.ascontiguousarray(xi.T).astype(bf)
        m["xnat"] = f32c(xi)
        in_maps.append(m)
    return in_maps


def kernel(x, ln_gamma, ln_beta, in_proj_w, in_proj_b, out_proj_w, out_proj_b):
    global LAST_RESULTS
    from concourse import bass_utils

    if "nc" not in _NC_CACHE:
        _NC_CACHE["nc"] = build_nc(S)
    nc = _NC_CACHE["nc"]

    in_maps = prep_inputs(x, ln_gamma, ln_beta, in_proj_w, in_proj_b,
                          out_proj_w, out_proj_b)
    res = bass_utils.run_bass_kernel_spmd(nc, in_maps, core_ids=list(range(N_CORES)))
    LAST_RESULTS = res
    out = np.stack([r["out"] for r in res.results], axis=0)
    return np.ascontiguousarray(out, dtype=np.float32)



# revision 8
# speedup vs baseline: 1.2112x; 1.2112x over previous
"""Self-contained Trainium2 Bass kernel: pre-LN multi-head attention block.

Computes, for x [B=8, S=1024, D=1024] (fp32) and packed attention weights:
    out = x + out_proj(MHA(LayerNorm(x)))
matching torch nn.MultiheadAttention's explicit (non-flash) path with 16 heads.

Sharding: data-parallel over batch - core i handles batch element i; no
collectives, outputs are concatenated on the host.

Per-core strategy (fp8 DoubleRow matmuls at 2x PE throughput):
  - LN runs on transposed activations (d on partitions); stats are matmuls
    against an all-ones stationary so the sums land partition-replicated in
    PSUM; the normalize chain runs in bf16 and writes xn directly in fp8.
  - QKV / V / PV / out-proj all run as fp8e4 DoubleRow matmuls: weights are
    pre-scaled by 32 on the host (power of two; folded back out via the
    softmax exp scale and the final output scale), each instruction
    contracts 2x128 d-coords at 0.5 cycles per output column.
  - scores^T[t,s] = K^T.T @ Q^T per head stay bf16 (K=64 contraction gains
    nothing from DoubleRow); exp runs on the scalar engine straight out of
    PSUM with scale 1/8192 and a -2 offset (cancels in softmax; keeps fp8
    exp outputs in range), writing fp8.
  - the softmax denominator comes from a DoubleRow matmul against an fp8
    all-ones stationary - its [64, N] output is the denominator replicated
    across 64 partitions, so the per-head normalize needs no broadcast:
    reciprocal + one tensor_tensor per s-half, writing ctx^T in fp8.
  - PE emission interleaves Q/K-projection units, scores, and PV+denominator
    per head pair so the scalar engine's exp stream (the second-busiest
    engine) overlaps PE work from ~15us onward.
  - residual + out_proj bias are pre-added on the host (bf16); the final
    merge is one fused scalar_tensor_tensor: (psum * 2^-10) + resid.
"""

import numpy as np
import ml_dtypes

P = 128
D = 1024
H = 16
DH = 64
B = 8
S = 1024
LN_EPS = 1e-5
N_CORES = 8

_ND = D // P   # d tiles (8)
NS = S // P    # s tiles (8)
NCK = 512      # LN chunk width
WS = 32.0      # fp8 weight pre-scale (power of two)
EXP_SCALE = 0.125 / (WS * WS)   # 1/8192: folds 1/sqrt(dh) and the q/k scales
EXP_BIAS = -3.0                 # cancels in softmax; keeps fp8 exp in range
OUT_SCALE = 1.0 / (WS * WS)     # folds the v/out-proj weight scales back out

LAST_RESULTS = None
_NC_CACHE = {}


def _emit(tc, aps):
    from concourse import mybir

    nc = tc.nc
    f32 = mybir.dt.float32
    bf16 = mybir.dt.bfloat16
    fp8 = mybir.dt.float8e4
    FT = mybir.ActivationFunctionType
    OP = mybir.AluOpType
    DR = mybir.MatmulPerfMode.DoubleRow

    xT, resid, wqkt, wvt, woutt, gammat, betat, binqk, binv, out = (
        aps["xt"], aps["resid"], aps["wqkt"], aps["wvt"], aps["woutt"],
        aps["gammat"], aps["betat"], aps["binqk"], aps["binv"], aps["out"],
    )

    with tc.tile_pool(name="consts", bufs=1) as consts, \
         tc.tile_pool(name="acts", bufs=1) as acts, \
         tc.tile_pool(name="wpool", bufs=1) as wpool:

        # ---------- constants (DMAs issued after the first x chunk) ----------
        cvec = consts.tile([P, 2 * _ND + H + 1], f32)
        gamma_sb = cvec[:, 0:_ND]
        beta_sb = cvec[:, _ND:2 * _ND]
        binqk_sb = cvec[:, 2 * _ND:2 * _ND + H]
        eps_sb = cvec[:, 2 * _ND + H:2 * _ND + H + 1]
        nc.vector.memset(eps_sb, LN_EPS)
        ones_mat = consts.tile([P, P], bf16)
        nc.vector.memset(ones_mat, 1.0)
        ones8 = consts.tile([P, P], fp8)
        nc.vector.memset(ones8, 1.0)
        expb = consts.tile([P, 1], f32)
        nc.vector.memset(expb, EXP_BIAS)
        ones8_pl = ones8.rearrange("p (a m) -> p a m", a=2)  # [P, 2, 64]
        binv_bc = consts.tile([P, D], f32)

        # ---------- persistent activations ----------
        xn8 = acts.tile([P, _ND, S], fp8)        # normalized x, transposed
        qkT = acts.tile([P, 2 * _ND, S], bf16)   # q tiles 0..7, k tiles 8..15
        v8 = acts.tile([P, NS, H, DH], fp8)      # v natural [t, h, dh]
        ctxT8 = acts.tile([P, _ND, S], fp8)      # normalized ctx^T (d on part)
        resid_sb = acts.tile([P, NS, D], bf16)   # x + out_proj_b, natural

        # ---------- weights (fp8, pre-scaled by WS on host) ----------
        wqk_sb = wpool.tile([P, _ND, 2 * D], fp8)
        wv_sb = wpool.tile([P, _ND, D], fp8)
        wout_sb = wpool.tile([P, _ND, D], fp8)

        # ================= Phase 1: LayerNorm =================
        with tc.tile_pool(name="lnsb", bufs=1) as lnsb, \
             tc.tile_pool(name="lnrow", bufs=1) as lnrow, \
             tc.tile_pool(name="lntmp", bufs=2) as lntmp, \
             tc.tile_pool(name="lnps", bufs=1, space="PSUM") as lnps:
            xT_sb = lnsb.tile([P, _ND, S], bf16)
            sx_ps = lnps.tile([P, S], f32, tag="sx")
            sx2_ps = lnps.tile([P, S], f32, tag="sx2")
            xT_r = xT.rearrange("(a p) s -> p a s", p=P)
            for c in range(S // NCK):
                sl = slice(c * NCK, (c + 1) * NCK)
                for j in range(_ND):
                    nc.sync.dma_start(out=xT_sb[:, j, sl], in_=xT_r[:, j, sl])
                if c == 0:
                    # everything else queues behind the first x chunk
                    nc.sync.dma_start(out=wqk_sb,
                                      in_=wqkt.rearrange("(a p) e -> p a e", p=P))
                    nc.sync.dma_start(out=cvec[:, 0:_ND], in_=gammat)
                    nc.sync.dma_start(out=cvec[:, _ND:2 * _ND], in_=betat)
                    nc.sync.dma_start(out=cvec[:, 2 * _ND:2 * _ND + H], in_=binqk)
                    nc.sync.dma_start(out=wv_sb,
                                      in_=wvt.rearrange("(a p) e -> p a e", p=P))
                    nc.sync.dma_start(out=wout_sb,
                                      in_=woutt.rearrange("(a p) e -> p a e", p=P))
                    nc.sync.dma_start(out=resid_sb,
                                      in_=resid.rearrange("(st p) e -> p st e", p=P))
                    nc.gpsimd.dma_start(out=binv_bc,
                                        in_=binv[None, :].to_broadcast((P, D)))
                for j in range(_ND):
                    sq = lntmp.tile([P, NCK], bf16, tag="sq", bufs=4)
                    with nc.allow_low_precision(reason="x^2 for LN stats in bf16"):
                        nc.vector.tensor_tensor(out=sq, in0=xT_sb[:, j, sl],
                                                in1=xT_sb[:, j, sl], op=OP.mult)
                    nc.tensor.matmul(sx_ps[:, sl], lhsT=ones_mat, rhs=xT_sb[:, j, sl],
                                     start=(j == 0), stop=(j == _ND - 1))
                    nc.tensor.matmul(sx2_ps[:, sl], lhsT=ones_mat, rhs=sq,
                                     start=(j == 0), stop=(j == _ND - 1))

                with nc.allow_low_precision(reason="LN stats chain in bf16"):
                    mu_bc = lnrow.tile([P, NCK], bf16, tag="mu", bufs=2)
                    nc.vector.tensor_scalar_mul(mu_bc, sx_ps[:, sl], 1.0 / D)
                    var_bc = lnrow.tile([P, NCK], f32, tag="var", bufs=2)
                    nc.vector.tensor_scalar_mul(var_bc, sx2_ps[:, sl], 1.0 / D)
                    musq = lnrow.tile([P, NCK], bf16, tag="musq", bufs=2)
                    nc.vector.tensor_tensor(out=musq, in0=mu_bc, in1=mu_bc, op=OP.mult)
                    nc.vector.tensor_tensor(out=var_bc, in0=var_bc, in1=musq,
                                            op=OP.subtract)
                    std_bc = lnrow.tile([P, NCK], bf16, tag="std", bufs=2)
                    nc.scalar.activation(out=std_bc, in_=var_bc, func=FT.Sqrt,
                                         bias=eps_sb)
                    b_bc = lnrow.tile([P, NCK], bf16, tag="b", bufs=2)
                    nc.vector.reciprocal(out=b_bc, in_=std_bc)
                    mub_bc = std_bc
                    nc.vector.tensor_tensor(out=mub_bc, in0=mu_bc, in1=b_bc,
                                            op=OP.mult)

                    for j in range(_ND):
                        t = lntmp.tile([P, NCK], bf16, tag="nrm", bufs=4)
                        nc.vector.tensor_tensor(out=t, in0=xT_sb[:, j, sl],
                                                in1=b_bc, op=OP.mult)
                        nc.vector.tensor_tensor(out=t, in0=t, in1=mub_bc,
                                                op=OP.subtract)
                        nc.scalar.activation(out=xn8[:, j, sl], in_=t,
                                             func=FT.Identity,
                                             bias=beta_sb[:, j:j + 1],
                                             scale=gamma_sb[:, j:j + 1])

        # ============ Phases 2-4: projections + attention + out-proj ========
        with tc.tile_pool(name="expool", bufs=1) as expool, \
             tc.tile_pool(name="sidep", bufs=1) as sidep, \
             tc.tile_pool(name="mps", bufs=1, space="PSUM") as mps:

            def dr_matmul(ps_out, lhsT, rhs, start, stop):
                nc.tensor.matmul(ps_out, lhsT=lhsT, rhs=rhs, start=start,
                                 stop=stop, perf_mode=DR)

            def emit_qk_unit(et):
                # e-tile et (128 cols of q|k): full-width [128, 512] DoubleRow
                # groups (stationary = [d, 2, 128] weight planes)
                for half in range(2):
                    ps = mps.tile([P, NCK], f32, tag="mm", bufs=2,
                                  name=f"qk{et}_{half}")
                    e0 = et * P
                    for jp in range(_ND // 2):
                        for c2 in range(2):
                            sl = slice(half * NCK + c2 * 256,
                                       half * NCK + (c2 + 1) * 256)
                            dr_matmul(
                                ps[:, c2 * 256:(c2 + 1) * 256],
                                wqk_sb[:, 2 * jp:2 * jp + 2, e0:e0 + P],
                                xn8[:, 2 * jp:2 * jp + 2, sl],
                                start=(jp == 0 and c2 == 0),
                                stop=(jp == _ND // 2 - 1 and c2 == 1))
                    sl = slice(half * NCK, (half + 1) * NCK)
                    with nc.allow_low_precision(reason="qk to bf16"):
                        nc.vector.tensor_scalar_add(qkT[:, et, sl], ps,
                                                    binqk_sb[:, et:et + 1])

            def emit_v_unit(st):
                # t-tile st: V natural [128 t, 512 e'] per e'-half
                # (stationary = [d, 2, 128] xn planes)
                for eh in range(2):
                    ps = mps.tile([P, NCK], f32, tag="mm", bufs=2,
                                  name=f"v{st}_{eh}")
                    t0 = st * P
                    for jp in range(_ND // 2):
                        for c2 in range(2):
                            sl = slice(eh * NCK + c2 * 256,
                                       eh * NCK + (c2 + 1) * 256)
                            dr_matmul(
                                ps[:, c2 * 256:(c2 + 1) * 256],
                                xn8[:, 2 * jp:2 * jp + 2, t0:t0 + P],
                                wv_sb[:, 2 * jp:2 * jp + 2, sl],
                                start=(jp == 0 and c2 == 0),
                                stop=(jp == _ND // 2 - 1 and c2 == 1))
                    with nc.allow_low_precision(reason="v to fp8"):
                        nc.vector.tensor_tensor(
                            out=v8[:, st, eh * 8:(eh + 1) * 8, :],
                            in0=ps.rearrange("p (h d) -> p h d", d=DH),
                            in1=binv_bc[:, eh * NCK:(eh + 1) * NCK]
                                .rearrange("p (h d) -> p h d", d=DH),
                            op=OP.add)

            def emit_scores(hp):
                # per head pair: scores^T then exp into fp8, [128, 512] tiles
                ex_t = expool.tile([P, 2, NS, S], fp8, tag="ex", bufs=2,
                                   name=f"ex{hp}")
                for tt in range(NS):
                    for idx in range(2):
                        base = idx * DH
                        for ch in range(2):
                            sl = slice(ch * NCK, (ch + 1) * NCK)
                            ps = mps.tile([P, NCK], f32, tag="sc", bufs=2,
                                          name=f"sc{hp}_{tt}_{idx}_{ch}")
                            nc.tensor.matmul(
                                ps,
                                lhsT=qkT[base:base + DH, 8 + hp, tt * P:(tt + 1) * P],
                                rhs=qkT[base:base + DH, hp, sl],
                                start=True, stop=True, tile_position=(base, 0))
                            with nc.allow_low_precision(reason="exp to fp8"):
                                nc.scalar.activation(out=ex_t[:, idx, tt, sl],
                                                     in_=ps, func=FT.Exp,
                                                     scale=EXP_SCALE, bias=expb)
                return ex_t

            def emit_pvden(hp, ex_t):
                # PV + denominator (DoubleRow, planes = t-tile pairs), then
                # normalize ctx^T in fp8.  den arrives replicated over 64
                # partitions so no broadcast is needed.  One [64, 512] psum
                # tile per head (matmul dst must sit at partition 0).
                for sh in range(2):
                    for idx in range(2):
                        h = 2 * hp + idx
                        ctxps = mps.tile([DH, NCK], f32, tag="ctx", bufs=2,
                                         name=f"ctx{hp}_{sh}_{idx}")
                        denps = mps.tile([DH, NCK], f32, tag="den", bufs=2,
                                         name=f"den{hp}_{sh}_{idx}")
                        for ttp in range(NS // 2):
                            for c2 in range(2):
                                sl = slice(sh * NCK + c2 * 256,
                                           sh * NCK + (c2 + 1) * 256)
                                co = slice(c2 * 256, (c2 + 1) * 256)
                                st_ = (ttp == 0 and c2 == 0)
                                sp_ = (ttp == NS // 2 - 1 and c2 == 1)
                                dr_matmul(ctxps[:, co],
                                          v8[:, 2 * ttp:2 * ttp + 2, h, :],
                                          ex_t[:, idx, 2 * ttp:2 * ttp + 2, sl],
                                          start=st_, stop=sp_)
                                dr_matmul(denps[:, co], ones8_pl,
                                          ex_t[:, idx, 2 * ttp:2 * ttp + 2, sl],
                                          start=st_, stop=sp_)
                        sl = slice(sh * NCK, (sh + 1) * NCK)
                        rden = sidep.tile([DH, NCK], bf16, tag="rd", bufs=4,
                                          name=f"rd{hp}_{sh}_{idx}")
                        with nc.allow_low_precision(reason="denom in bf16"):
                            nc.vector.reciprocal(out=rden, in_=denps)
                            nc.vector.tensor_tensor(
                                out=ctxT8[idx * DH:(idx + 1) * DH, hp, sl],
                                in0=ctxps, in1=rden, op=OP.mult)

            def emit_outproj():
                for st in range(NS):
                    for eh in range(2):
                        ps = mps.tile([P, NCK], f32, tag="mm", bufs=2,
                                      name=f"op{st}_{eh}")
                        s0 = st * P
                        for hpp in range(_ND // 2):
                            for c2 in range(2):
                                sl = slice(eh * NCK + c2 * 256,
                                           eh * NCK + (c2 + 1) * 256)
                                dr_matmul(
                                    ps[:, c2 * 256:(c2 + 1) * 256],
                                    ctxT8[:, 2 * hpp:2 * hpp + 2, s0:s0 + P],
                                    wout_sb[:, 2 * hpp:2 * hpp + 2, sl],
                                    start=(hpp == 0 and c2 == 0),
                                    stop=(hpp == _ND // 2 - 1 and c2 == 1))
                        sl = slice(eh * NCK, (eh + 1) * NCK)
                        ot = sidep.tile([P, NCK], f32, tag="ot", bufs=4,
                                        name=f"ot{st}_{eh}")
                        nc.vector.scalar_tensor_tensor(
                            out=ot, in0=ps, scalar=OUT_SCALE,
                            in1=resid_sb[:, st, sl],
                            op0=OP.mult, op1=OP.add)
                        nc.sync.dma_start(out=out[st * P:(st + 1) * P, sl], in_=ot)

            # ---- interleaved emission: qk(p+1) | scores(p) | pvden(p-1) ----
            emit_qk_unit(0)
            emit_qk_unit(8)
            ex_prev = emit_scores(0)
            emit_qk_unit(1)
            emit_qk_unit(9)
            ex_cur = emit_scores(1)
            for st in range(NS):
                emit_v_unit(st)
            emit_pvden(0, ex_prev)
            ex_prev = ex_cur
            for p in range(2, H // 2):
                emit_qk_unit(p)
                emit_qk_unit(8 + p)
                ex_cur = emit_scores(p)
                emit_pvden(p - 1, ex_prev)
                ex_prev = ex_cur
            emit_pvden(H // 2 - 1, ex_prev)
            emit_outproj()


def build_nc():
    import concourse.bacc as bacc
    import concourse.tile as tile
    from concourse import mybir

    f32 = mybir.dt.float32
    bf16 = mybir.dt.bfloat16
    fp8 = mybir.dt.float8e4

    nc = bacc.Bacc("TRN2", target_bir_lowering=False, debug=False)
    aps = {
        "xt": nc.dram_tensor("xt", [D, S], bf16, kind="ExternalInput").ap(),
        "resid": nc.dram_tensor("resid", [S, D], bf16, kind="ExternalInput").ap(),
        "wqkt": nc.dram_tensor("wqkt", [D, 2 * D], fp8, kind="ExternalInput").ap(),
        "wvt": nc.dram_tensor("wvt", [D, D], fp8, kind="ExternalInput").ap(),
        "woutt": nc.dram_tensor("woutt", [D, D], fp8, kind="ExternalInput").ap(),
        "gammat": nc.dram_tensor("gammat", [P, _ND], f32, kind="ExternalInput").ap(),
        "betat": nc.dram_tensor("betat", [P, _ND], f32, kind="ExternalInput").ap(),
        "binqk": nc.dram_tensor("binqk", [P, H], f32, kind="ExternalInput").ap(),
        "binv": nc.dram_tensor("binv", [D], f32, kind="ExternalInput").ap(),
        "out": nc.dram_tensor("out", [S, D], f32, kind="ExternalOutput").ap(),
    }
    with tile.TileContext(nc) as tc:
        _emit(tc, aps)
    nc.compile()
    return nc


def prep_inputs(x, ln_gamma, ln_beta, in_proj_w, in_proj_b, out_proj_w, out_proj_b,
                n_cores=N_CORES):
    bf = ml_dtypes.bfloat16
    f8 = ml_dtypes.float8_e4m3
    f32c = lambda a: np.ascontiguousarray(a, dtype=np.float32)
    win = np.asarray(in_proj_w, np.float32)
    shared = {
        "wqkt": np.ascontiguousarray((win[:2 * D] * WS).T).astype(f8),
        "wvt": np.ascontiguousarray((win[2 * D:] * WS).T).astype(f8),
        "woutt": np.ascontiguousarray(np.asarray(out_proj_w, np.float32).T * WS).astype(f8),
        "gammat": f32c(np.asarray(ln_gamma, np.float32).reshape(_ND, P).T),
        "betat": f32c(np.asarray(ln_beta, np.float32).reshape(_ND, P).T),
        "binqk": f32c((np.asarray(in_proj_b, np.float32)[:2 * D] * WS)
                      .reshape(H, P).T),
        "binv": f32c(np.asarray(in_proj_b, np.float32)[2 * D:] * WS),
    }
    bout = np.asarray(out_proj_b, np.float32)
    in_maps = []
    for i in range(n_cores):
        xi = np.asarray(x[i], np.float32)
        m = dict(shared)
        m["xt"] = np.ascontiguousarray(xi.T).astype(bf)
        m["resid"] = np.ascontiguousarray(xi + bout).astype(bf)
        in_maps.append(m)
    return in_maps


def kernel(x, ln_gamma, ln_beta, in_proj_w, in_proj_b, out_proj_w, out_proj_b):
    global LAST_RESULTS
    from concourse import bass_utils

    if "nc" not in _NC_CACHE:
        _NC_CACHE["nc"] = build_nc()
    nc = _NC_CACHE["nc"]

    in_maps = prep_inputs(x, ln_gamma, ln_beta, in_proj_w, in_proj_b,
                          out_proj_w, out_proj_b)
    res = bass_utils.run_bass_kernel_spmd(nc, in_maps, core_ids=list(range(N_CORES)))
    LAST_RESULTS = res
    out = np.stack([r["out"] for r in res.results], axis=0)
    return np.ascontiguousarray(out, dtype=np.float32)


# revision 11
# speedup vs baseline: 1.2736x; 1.0515x over previous
"""Self-contained Trainium2 Bass kernel: pre-LN multi-head attention block.

Computes, for x [B=8, S=1024, D=1024] (fp32) and packed attention weights:
    out = x + out_proj(MHA(LayerNorm(x)))
matching torch nn.MultiheadAttention's explicit (non-flash) path with 16 heads.

Sharding: data-parallel over batch - core i handles batch element i; no
collectives, outputs are concatenated on the host.

Per-core strategy (fp8 DoubleRow matmuls at 2x PE throughput):
  - LN runs on transposed activations (d on partitions); stats are matmuls
    against an all-ones stationary so the sums land partition-replicated in
    PSUM; the normalize chain runs in bf16 on DVE and the gamma/beta apply
    runs on the Pool engine (tensor_scalar), writing xn directly in fp8.
  - QKV / V / PV / out-proj all run as fp8e4 DoubleRow matmuls with full
    128-wide stationaries: weights are pre-scaled by 32 on the host (power
    of two; folded back out via the softmax exp scale and the final output
    scale); each instruction contracts 2x128 d-coords at 0.5 cycles per
    output column.
  - scores^T[t,s] = K^T.T @ Q^T per head stay bf16 (K=64 contraction gains
    nothing from DoubleRow); exp runs on the scalar engine over [128, 1024]
    PSUM tiles (amortizing the fixed ACT access latency) with scale 1/8192
    and a -3 offset (cancels in softmax; keeps fp8 exp in range).
  - the softmax denominator comes from a DoubleRow matmul against an fp8
    all-ones stationary - its [64, N] output is the denominator replicated
    across 64 partitions, so the per-head normalize needs no broadcast.
  - PE emission: Q/K/V units are split by s-half so the first halves (plus
    warmup matmuls) keep the PE busy while LayerNorm finishes the second
    x chunk; per head pair the stream is qk(p+1) | scores(p) | pv+den(p-1)
    so the scalar engine's exp stream overlaps PE work throughout.
  - PSUM->SBUF copies alternate between DVE and the otherwise-idle Pool
    engine; residual + out_proj bias are pre-added on the host (bf16) and
    merged with one fused scalar_tensor_tensor: (psum * 2^-10) + resid.
"""

import numpy as np
import ml_dtypes

P = 128
D = 1024
H = 16
DH = 64
B = 8
S = 1024
LN_EPS = 1e-5
N_CORES = 8

_ND = D // P   # d tiles (8)
NS = S // P    # s tiles (8)
NCK = 512      # LN chunk / matmul moving width
WS = 32.0      # fp8 weight pre-scale (power of two)
EXP_SCALE = 0.125 / (WS * WS)   # 1/8192: folds 1/sqrt(dh) and the q/k scales
EXP_BIAS = -3.0                 # cancels in softmax; keeps fp8 exp in range
OUT_SCALE = 1.0 / (WS * WS)     # folds the v/out-proj weight scales back out

LAST_RESULTS = None
_NC_CACHE = {}


def _emit(tc, aps):
    from concourse import mybir

    nc = tc.nc
    f32 = mybir.dt.float32
    bf16 = mybir.dt.bfloat16
    fp8 = mybir.dt.float8e4
    FT = mybir.ActivationFunctionType
    OP = mybir.AluOpType
    DR = mybir.MatmulPerfMode.DoubleRow

    xT, resid, wqkt, wvt, woutt, gammat, betat, binqk, binv, out = (
        aps["xt"], aps["resid"], aps["wqkt"], aps["wvt"], aps["woutt"],
        aps["gammat"], aps["betat"], aps["binqk"], aps["binv"], aps["out"],
    )

    with tc.tile_pool(name="consts", bufs=1) as consts, \
         tc.tile_pool(name="acts", bufs=1) as acts, \
         tc.tile_pool(name="wpool", bufs=1) as wpool:

        # ---------- constants (DMAs issued after the first x chunk) ----------
        cvec = consts.tile([P, 2 * _ND + H + 1], f32)
        gamma_sb = cvec[:, 0:_ND]
        beta_sb = cvec[:, _ND:2 * _ND]
        binqk_sb = cvec[:, 2 * _ND:2 * _ND + H]
        eps_sb = cvec[:, 2 * _ND + H:2 * _ND + H + 1]
        nc.vector.memset(eps_sb, LN_EPS)
        ones_mat = consts.tile([P, P], bf16)
        nc.vector.memset(ones_mat, 1.0)
        ones8 = consts.tile([P, P], fp8)
        nc.vector.memset(ones8, 1.0)
        ones8_pl = ones8.rearrange("p (a m) -> p a m", a=2)  # [P, 2, 64]
        expb = consts.tile([P, 1], f32)
        nc.vector.memset(expb, EXP_BIAS)
        binv_bc = consts.tile([P, D], f32)

        # ---------- persistent activations ----------
        xn8 = acts.tile([P, _ND, S], fp8)        # normalized x, transposed
        qkT = acts.tile([P, 2 * _ND, S], bf16)   # q tiles 0..7, k tiles 8..15
        v8 = acts.tile([P, NS, H, DH], fp8)      # v natural [t, h, dh]
        ctxT8 = acts.tile([P, _ND, S], fp8)      # normalized ctx^T (d on part)
        resid_sb = acts.tile([P, NS, D], bf16)   # x + out_proj_b, natural

        # ---------- weights (fp8, pre-scaled by WS on host) ----------
        wqk_sb = wpool.tile([P, _ND, 2 * D], fp8)
        wv_sb = wpool.tile([P, _ND, D], fp8)
        wout_sb = wpool.tile([P, _ND, D], fp8)

        # ================= Phase 1: LayerNorm =================
        with tc.tile_pool(name="lnsb", bufs=1) as lnsb, \
             tc.tile_pool(name="lnrow", bufs=1) as lnrow, \
             tc.tile_pool(name="lntmp", bufs=2) as lntmp, \
             tc.tile_pool(name="lnps", bufs=1, space="PSUM") as lnps:
            xT_sb = lnsb.tile([P, _ND, S], bf16)
            sx_ps = lnps.tile([P, S], f32, tag="sx")
            sx2_ps = lnps.tile([P, S], f32, tag="sx2")
            # PE p-state warmup: dummy matmuls chain into the LN stats so the
            # clock is at full speed (and stays there) when real work arrives
            warm_ps = lnps.tile([P, P], f32, tag="warm")
            for _ in range(24):
                nc.tensor.matmul(warm_ps, lhsT=ones_mat, rhs=ones_mat,
                                 start=True, stop=True)
            xT_r = xT.rearrange("(a p) s -> p a s", p=P)
            for c in range(S // NCK):
                sl = slice(c * NCK, (c + 1) * NCK)
                for j in range(_ND):
                    nc.sync.dma_start(out=xT_sb[:, j, sl], in_=xT_r[:, j, sl])
                if c == 0:
                    # everything else queues behind the first x chunk
                    nc.sync.dma_start(out=wqk_sb,
                                      in_=wqkt.rearrange("(a p) e -> p a e", p=P))
                    nc.sync.dma_start(out=cvec[:, 0:_ND], in_=gammat)
                    nc.sync.dma_start(out=cvec[:, _ND:2 * _ND], in_=betat)
                    nc.sync.dma_start(out=cvec[:, 2 * _ND:2 * _ND + H], in_=binqk)
                    nc.sync.dma_start(out=wv_sb,
                                      in_=wvt.rearrange("(a p) e -> p a e", p=P))
                    nc.sync.dma_start(out=wout_sb,
                                      in_=woutt.rearrange("(a p) e -> p a e", p=P))
                    nc.sync.dma_start(out=resid_sb,
                                      in_=resid.rearrange("(st p) e -> p st e", p=P))
                    nc.gpsimd.dma_start(out=binv_bc,
                                        in_=binv[None, :].to_broadcast((P, D)))
                for j in range(_ND):
                    sq = lntmp.tile([P, NCK], bf16, tag="sq", bufs=4)
                    with nc.allow_low_precision(reason="x^2 for LN stats in bf16"):
                        nc.vector.tensor_tensor(out=sq, in0=xT_sb[:, j, sl],
                                                in1=xT_sb[:, j, sl], op=OP.mult)
                    nc.tensor.matmul(sx_ps[:, sl], lhsT=ones_mat, rhs=xT_sb[:, j, sl],
                                     start=(j == 0), stop=(j == _ND - 1))
                    nc.tensor.matmul(sx2_ps[:, sl], lhsT=ones_mat, rhs=sq,
                                     start=(j == 0), stop=(j == _ND - 1))

                with nc.allow_low_precision(reason="LN stats chain in bf16"):
                    mu_bc = lnrow.tile([P, NCK], bf16, tag="mu", bufs=2)
                    nc.vector.tensor_scalar_mul(mu_bc, sx_ps[:, sl], 1.0 / D)
                    var_bc = lnrow.tile([P, NCK], f32, tag="var", bufs=2)
                    nc.vector.tensor_scalar_mul(var_bc, sx2_ps[:, sl], 1.0 / D)
                    musq = lnrow.tile([P, NCK], bf16, tag="musq", bufs=2)
                    nc.vector.tensor_tensor(out=musq, in0=mu_bc, in1=mu_bc, op=OP.mult)
                    nc.vector.tensor_tensor(out=var_bc, in0=var_bc, in1=musq,
                                            op=OP.subtract)
                    std_bc = lnrow.tile([P, NCK], bf16, tag="std", bufs=2)
                    nc.scalar.activation(out=std_bc, in_=var_bc, func=FT.Sqrt,
                                         bias=eps_sb)
                    b_bc = lnrow.tile([P, NCK], bf16, tag="b", bufs=2)
                    nc.vector.reciprocal(out=b_bc, in_=std_bc)
                    mub_bc = std_bc
                    nc.vector.tensor_tensor(out=mub_bc, in0=mu_bc, in1=b_bc,
                                            op=OP.mult)

                    for j in range(_ND):
                        t = lntmp.tile([P, NCK], bf16, tag="nrm", bufs=4)
                        eng = nc.vector if j % 2 == 0 else nc.gpsimd
                        eng.tensor_tensor(out=t, in0=xT_sb[:, j, sl],
                                          in1=b_bc, op=OP.mult)
                        eng.tensor_tensor(out=t, in0=t, in1=mub_bc,
                                          op=OP.subtract)
                        # gamma/beta apply on the Pool engine, fp8 out
                        nc.gpsimd.tensor_scalar(out=xn8[:, j, sl], in0=t,
                                                scalar1=gamma_sb[:, j:j + 1],
                                                scalar2=beta_sb[:, j:j + 1],
                                                op0=OP.mult, op1=OP.add)

        # ============ Phases 2-4: projections + attention + out-proj ========
        with tc.tile_pool(name="expool", bufs=1) as expool, \
             tc.tile_pool(name="sidep", bufs=1) as sidep, \
             tc.tile_pool(name="mps", bufs=1, space="PSUM") as mps:

            def dr_matmul(ps_out, lhsT, rhs, start, stop):
                nc.tensor.matmul(ps_out, lhsT=lhsT, rhs=rhs, start=start,
                                 stop=stop, perf_mode=DR)

            def veng(i):
                return nc.vector if i % 2 == 0 else nc.gpsimd

            def emit_qk_half(et, half):
                # e-tile et (128 cols of q|k), s-half: one [128, 512] group
                ps = mps.tile([P, NCK], f32, tag="mm", bufs=2,
                              name=f"qk{et}_{half}")
                e0 = et * P
                for jp in range(_ND // 2):
                    for c2 in range(2):
                        sl = slice(half * NCK + c2 * 256,
                                   half * NCK + (c2 + 1) * 256)
                        dr_matmul(
                            ps[:, c2 * 256:(c2 + 1) * 256],
                            wqk_sb[:, 2 * jp:2 * jp + 2, e0:e0 + P],
                            xn8[:, 2 * jp:2 * jp + 2, sl],
                            start=(jp == 0 and c2 == 0),
                            stop=(jp == _ND // 2 - 1 and c2 == 1))
                sl = slice(half * NCK, (half + 1) * NCK)
                with nc.allow_low_precision(reason="qk to bf16"):
                    nc.vector.tensor_scalar_add(qkT[:, et, sl], ps,
                                                binqk_sb[:, et:et + 1])

            def emit_v_unit(st):
                # t-tile st: V natural [128 t, 512 e'] per e'-half
                for eh in range(2):
                    ps = mps.tile([P, NCK], f32, tag="mm", bufs=2,
                                  name=f"v{st}_{eh}")
                    t0 = st * P
                    for jp in range(_ND // 2):
                        for c2 in range(2):
                            sl = slice(eh * NCK + c2 * 256,
                                       eh * NCK + (c2 + 1) * 256)
                            dr_matmul(
                                ps[:, c2 * 256:(c2 + 1) * 256],
                                xn8[:, 2 * jp:2 * jp + 2, t0:t0 + P],
                                wv_sb[:, 2 * jp:2 * jp + 2, sl],
                                start=(jp == 0 and c2 == 0),
                                stop=(jp == _ND // 2 - 1 and c2 == 1))
                    with nc.allow_low_precision(reason="v to fp8"):
                        nc.vector.tensor_tensor(
                            out=v8[:, st, eh * 8:(eh + 1) * 8, :],
                            in0=ps.rearrange("p (h d) -> p h d", d=DH),
                            in1=binv_bc[:, eh * NCK:(eh + 1) * NCK]
                                .rearrange("p (h d) -> p h d", d=DH),
                            op=OP.add)

            def emit_scores(hp):
                # per head pair: scores^T into [128, 1024] psum tiles, then a
                # single wide exp (fp8 out) per (tt, idx)
                ex_t = expool.tile([P, 2, NS, S], fp8, tag="ex", bufs=2,
                                   name=f"ex{hp}")
                for tt in range(NS):
                    for idx in range(2):
                        base = idx * DH
                        ps = mps.tile([P, S], f32, tag="sc", bufs=2,
                                      name=f"sc{hp}_{tt}_{idx}")
                        for sh in range(2):
                            sl = slice(sh * NCK, (sh + 1) * NCK)
                            nc.tensor.matmul(
                                ps[:, sl],
                                lhsT=qkT[base:base + DH, 8 + hp, tt * P:(tt + 1) * P],
                                rhs=qkT[base:base + DH, hp, sl],
                                start=True, stop=True, tile_position=(base, 0))
                        with nc.allow_low_precision(reason="exp to fp8"):
                            nc.scalar.activation(out=ex_t[:, idx, tt, :],
                                                 in_=ps, func=FT.Exp,
                                                 scale=EXP_SCALE, bias=expb)
                return ex_t

            def emit_pvden(hp, ex_t):
                # PV + denominator (DoubleRow, planes = t-tile pairs), then
                # normalize ctx^T in fp8.  den rides the "mm" psum tag and
                # arrives replicated over 64 partitions (no broadcast needed).
                for sh in range(2):
                    for idx in range(2):
                        h = 2 * hp + idx
                        ctxps = mps.tile([DH, NCK], f32, tag="ctx", bufs=2,
                                         name=f"ctx{hp}_{sh}_{idx}")
                        denft = mps.tile([P, NCK], f32, tag="mm", bufs=2,
                                         name=f"den{hp}_{sh}_{idx}")
                        denps = denft[0:DH, :]
                        for ttp in range(NS // 2):
                            for c2 in range(2):
                                sl = slice(sh * NCK + c2 * 256,
                                           sh * NCK + (c2 + 1) * 256)
                                co = slice(c2 * 256, (c2 + 1) * 256)
                                st_ = (ttp == 0 and c2 == 0)
                                sp_ = (ttp == NS // 2 - 1 and c2 == 1)
                                dr_matmul(ctxps[:, co],
                                          v8[:, 2 * ttp:2 * ttp + 2, h, :],
                                          ex_t[:, idx, 2 * ttp:2 * ttp + 2, sl],
                                          start=st_, stop=sp_)
                                dr_matmul(denps[:, co], ones8_pl,
                                          ex_t[:, idx, 2 * ttp:2 * ttp + 2, sl],
                                          start=st_, stop=sp_)
                        sl = slice(sh * NCK, (sh + 1) * NCK)
                        rden = sidep.tile([DH, NCK], bf16, tag="rd", bufs=4,
                                          name=f"rd{hp}_{sh}_{idx}")
                        with nc.allow_low_precision(reason="denom in bf16"):
                            nc.vector.reciprocal(out=rden, in_=denps)
                            nc.vector.tensor_tensor(
                                out=ctxT8[idx * DH:(idx + 1) * DH, hp, sl],
                                in0=ctxps, in1=rden, op=OP.mult)

            def emit_outproj():
                for st in range(NS):
                    for eh in range(2):
                        ps = mps.tile([P, NCK], f32, tag="mm", bufs=2,
                                      name=f"op{st}_{eh}")
                        s0 = st * P
                        for hpp in range(_ND // 2):
                            for c2 in range(2):
                                sl = slice(eh * NCK + c2 * 256,
                                           eh * NCK + (c2 + 1) * 256)
                                dr_matmul(
                                    ps[:, c2 * 256:(c2 + 1) * 256],
                                    ctxT8[:, 2 * hpp:2 * hpp + 2, s0:s0 + P],
                                    wout_sb[:, 2 * hpp:2 * hpp + 2, sl],
                                    start=(hpp == 0 and c2 == 0),
                                    stop=(hpp == _ND // 2 - 1 and c2 == 1))
                        sl = slice(eh * NCK, (eh + 1) * NCK)
                        ot = sidep.tile([P, NCK], f32, tag="ot", bufs=4,
                                        name=f"ot{st}_{eh}")
                        nc.vector.scalar_tensor_tensor(
                            out=ot, in0=ps, scalar=OUT_SCALE,
                            in1=resid_sb[:, st, sl],
                            op0=OP.mult, op1=OP.add)
                        nc.sync.dma_start(out=out[st * P:(st + 1) * P, sl], in_=ot)

            # ---- interleaved emission ----
            # s-half-0 work first: runs while LayerNorm's second chunk is
            # still on DVE, keeping the PE busy
            for et in range(2 * _ND):
                emit_qk_half(et, 0)
            for st in range(NS // 2):
                emit_v_unit(st)
            # pair 0/1 second halves, then steady-state interleave
            emit_qk_half(0, 1)
            emit_qk_half(8, 1)
            ex_prev = emit_scores(0)
            emit_qk_half(1, 1)
            emit_qk_half(9, 1)
            ex_cur = emit_scores(1)
            for st in range(NS // 2, NS):
                emit_v_unit(st)
            emit_pvden(0, ex_prev)
            ex_prev = ex_cur
            for p in range(2, H // 2):
                emit_qk_half(p, 1)
                emit_qk_half(8 + p, 1)
                ex_cur = emit_scores(p)
                emit_pvden(p - 1, ex_prev)
                ex_prev = ex_cur
            emit_pvden(H // 2 - 1, ex_prev)
            emit_outproj()


def build_nc():
    import concourse.bacc as bacc
    import concourse.tile as tile
    from concourse import mybir

    f32 = mybir.dt.float32
    bf16 = mybir.dt.bfloat16
    fp8 = mybir.dt.float8e4

    nc = bacc.Bacc("TRN2", target_bir_lowering=False, debug=False)
    aps = {
        "xt": nc.dram_tensor("xt", [D, S], bf16, kind="ExternalInput").ap(),
        "resid": nc.dram_tensor("resid", [S, D], bf16, kind="ExternalInput").ap(),
        "wqkt": nc.dram_tensor("wqkt", [D, 2 * D], fp8, kind="ExternalInput").ap(),
        "wvt": nc.dram_tensor("wvt", [D, D], fp8, kind="ExternalInput").ap(),
        "woutt": nc.dram_tensor("woutt", [D, D], fp8, kind="ExternalInput").ap(),
        "gammat": nc.dram_tensor("gammat", [P, _ND], f32, kind="ExternalInput").ap(),
        "betat": nc.dram_tensor("betat", [P, _ND], f32, kind="ExternalInput").ap(),
        "binqk": nc.dram_tensor("binqk", [P, H], f32, kind="ExternalInput").ap(),
        "binv": nc.dram_tensor("binv", [D], f32, kind="ExternalInput").ap(),
        "out": nc.dram_tensor("out", [S, D], f32, kind="ExternalOutput").ap(),
    }
    with tile.TileContext(nc) as tc:
        _emit(tc, aps)
    nc.compile()
    return nc


def prep_inputs(x, ln_gamma, ln_beta, in_proj_w, in_proj_b, out_proj_w, out_proj_b,
                n_cores=N_CORES):
    bf = ml_dtypes.bfloat16
    f8 = ml_dtypes.float8_e4m3
    f32c = lambda a: np.ascontiguousarray(a, dtype=np.float32)
    win = np.asarray(in_proj_w, np.float32)
    shared = {
        "wqkt": np.ascontiguousarray((win[:2 * D] * WS).T).astype(f8),
        "wvt": np.ascontiguousarray((win[2 * D:] * WS).T).astype(f8),
        "woutt": np.ascontiguousarray(np.asarray(out_proj_w, np.float32).T * WS).astype(f8),
        "gammat": f32c(np.asarray(ln_gamma, np.float32).reshape(_ND, P).T),
        "betat": f32c(np.asarray(ln_beta, np.float32).reshape(_ND, P).T),
        "binqk": f32c((np.asarray(in_proj_b, np.float32)[:2 * D] * WS)
                      .reshape(H, P).T),
        "binv": f32c(np.asarray(in_proj_b, np.float32)[2 * D:] * WS),
    }
    bout = np.asarray(out_proj_b, np.float32)
    in_maps = []
    for i in range(n_cores):
        xi = np.asarray(x[i], np.float32)
        m = dict(shared)
        m["xt"] = np.ascontiguousarray(xi.T).astype(bf)
        m["resid"] = np.ascontiguousarray(xi + bout).astype(bf)
        in_maps.append(m)
    return in_maps


def kernel(x, ln_gamma, ln_beta, in_proj_w, in_proj_b, out_proj_w, out_proj_b):
    global LAST_RESULTS
    from concourse import bass_utils

    if "nc" not in _NC_CACHE:
        _NC_CACHE["nc"] = build_nc()
    nc = _NC_CACHE["nc"]

    in_maps = prep_inputs(x, ln_gamma, ln_beta, in_proj_w, in_proj_b,
                          out_proj_w, out_proj_b)
    res = bass_utils.run_bass_kernel_spmd(nc, in_maps, core_ids=list(range(N_CORES)))
    LAST_RESULTS = res
    out = np.stack([r["out"] for r in res.results], axis=0)
    return np.ascontiguousarray(out, dtype=np.float32)


# revision 12
# speedup vs baseline: 1.3081x; 1.0271x over previous
"""Self-contained Trainium2 Bass kernel: pre-LN multi-head attention block.

Computes, for x [B=8, S=1024, D=1024] (fp32) and packed attention weights:
    out = x + out_proj(MHA(LayerNorm(x)))
matching torch nn.MultiheadAttention's explicit (non-flash) path with 16 heads.

Sharding: data-parallel over batch - core i handles batch element i; no
collectives, outputs are concatenated on the host.

Per-core strategy (fp8 DoubleRow matmuls at 2x PE throughput):
  - LN runs on transposed activations (d on partitions); stats are matmuls
    against an all-ones stationary so the sums land partition-replicated in
    PSUM; the normalize chain runs in bf16 on DVE and the gamma/beta apply
    runs on the Pool engine (tensor_scalar), writing xn directly in fp8.
  - QKV / V / PV / out-proj all run as fp8e4 DoubleRow matmuls with full
    128-wide stationaries: weights are pre-scaled by 32 on the host (power
    of two; folded back out via the softmax exp scale and the final output
    scale); each instruction contracts 2x128 d-coords at 0.5 cycles per
    output column.
  - scores^T[t,s] = K^T.T @ Q^T per head stay bf16 (K=64 contraction gains
    nothing from DoubleRow); exp runs on the scalar engine over [128, 1024]
    PSUM tiles (amortizing the fixed ACT access latency) with scale 1/8192
    and a -3 offset (cancels in softmax; keeps fp8 exp in range).
  - the softmax denominator comes from a DoubleRow matmul against an fp8
    all-ones stationary - its [64, N] output is the denominator replicated
    across 64 partitions, so the per-head normalize needs no broadcast.
  - PE emission: Q/K/V units are split by s-half so the first halves (plus
    warmup matmuls) keep the PE busy while LayerNorm finishes the second
    x chunk; per head pair the stream is qk(p+1) | scores(p) | pv+den(p-1)
    so the scalar engine's exp stream overlaps PE work throughout.
  - PSUM->SBUF copies alternate between DVE and the otherwise-idle Pool
    engine; residual + out_proj bias are pre-added on the host (bf16) and
    merged with one fused scalar_tensor_tensor: (psum * 2^-10) + resid.
"""

import numpy as np
import ml_dtypes

P = 128
D = 1024
H = 16
DH = 64
B = 8
S = 1024
LN_EPS = 1e-5
N_CORES = 8

_ND = D // P   # d tiles (8)
NS = S // P    # s tiles (8)
NCK = 512      # LN chunk / matmul moving width
WS = 32.0      # fp8 weight pre-scale (power of two)
EXP_SCALE = 0.125 / (WS * WS)   # 1/8192: folds 1/sqrt(dh) and the q/k scales
EXP_BIAS = -3.0                 # cancels in softmax; keeps fp8 exp in range
OUT_SCALE = 1.0 / (WS * WS)     # folds the v/out-proj weight scales back out

LAST_RESULTS = None
_NC_CACHE = {}


def _emit(tc, aps):
    from concourse import mybir

    nc = tc.nc
    f32 = mybir.dt.float32
    bf16 = mybir.dt.bfloat16
    fp8 = mybir.dt.float8e4
    FT = mybir.ActivationFunctionType
    OP = mybir.AluOpType
    DR = mybir.MatmulPerfMode.DoubleRow

    xT, resid, wqkt, wvt, woutt, binqk, binv, out = (
        aps["xt"], aps["resid"], aps["wqkt"], aps["wvt"], aps["woutt"],
        aps["binqk"], aps["binv"], aps["out"],
    )

    with tc.tile_pool(name="consts", bufs=1) as consts, \
         tc.tile_pool(name="acts", bufs=1) as acts, \
         tc.tile_pool(name="wpool", bufs=1) as wpool:

        # ---------- constants (DMAs issued after the first x chunk) ----------
        cvec = consts.tile([P, H + 1], f32)
        binqk_sb = cvec[:, 0:H]
        eps_sb = cvec[:, H:H + 1]
        nc.vector.memset(eps_sb, LN_EPS)
        ones_mat = consts.tile([P, P], bf16)
        nc.vector.memset(ones_mat, 1.0)
        ones8 = consts.tile([P, P], fp8)
        nc.vector.memset(ones8, 1.0)
        ones8_pl = ones8.rearrange("p (a m) -> p a m", a=2)  # [P, 2, 64]
        expb = consts.tile([P, 1], f32)
        nc.vector.memset(expb, EXP_BIAS)
        binv_bc = consts.tile([P, D], f32)

        # ---------- persistent activations ----------
        xn8 = acts.tile([P, _ND, S], fp8)        # normalized x, transposed
        qkT = acts.tile([P, 2 * _ND, S], bf16)   # q tiles 0..7, k tiles 8..15
        v8 = acts.tile([P, NS, H, DH], fp8)      # v natural [t, h, dh]
        ctxT8 = acts.tile([P, _ND, S], fp8)      # normalized ctx^T (d on part)
        resid_sb = acts.tile([P, NS, D], bf16)   # x + out_proj_b, natural

        # ---------- weights (fp8, pre-scaled by WS on host) ----------
        wqk_sb = wpool.tile([P, _ND, 2 * D], fp8)
        wv_sb = wpool.tile([P, _ND, D], fp8)
        wout_sb = wpool.tile([P, _ND, D], fp8)

        # ================= Phase 1: LayerNorm =================
        with tc.tile_pool(name="lnsb", bufs=1) as lnsb, \
             tc.tile_pool(name="lnrow", bufs=1) as lnrow, \
             tc.tile_pool(name="lntmp", bufs=2) as lntmp, \
             tc.tile_pool(name="lnps", bufs=1, space="PSUM") as lnps:
            xT_sb = lnsb.tile([P, _ND, S], bf16)
            sx_ps = lnps.tile([P, S], f32, tag="sx")
            sx2_ps = lnps.tile([P, S], f32, tag="sx2")
            # PE p-state warmup: dummy matmuls chain into the LN stats so the
            # clock is at full speed (and stays there) when real work arrives
            warm_ps = lnps.tile([P, P], f32, tag="warm")
            for _ in range(24):
                nc.tensor.matmul(warm_ps, lhsT=ones_mat, rhs=ones_mat,
                                 start=True, stop=True)
            xT_r = xT.rearrange("(a p) s -> p a s", p=P)
            for c in range(S // NCK):
                sl = slice(c * NCK, (c + 1) * NCK)
                for j in range(_ND):
                    nc.sync.dma_start(out=xT_sb[:, j, sl], in_=xT_r[:, j, sl])
                if c == 0:
                    # everything else queues behind the first x chunk
                    nc.sync.dma_start(out=wqk_sb,
                                      in_=wqkt.rearrange("(a p) e -> p a e", p=P))
                    nc.sync.dma_start(out=cvec[:, 0:H], in_=binqk)
                    nc.sync.dma_start(out=wv_sb,
                                      in_=wvt.rearrange("(a p) e -> p a e", p=P))
                    nc.sync.dma_start(out=wout_sb,
                                      in_=woutt.rearrange("(a p) e -> p a e", p=P))
                    nc.sync.dma_start(out=resid_sb,
                                      in_=resid.rearrange("(st p) e -> p st e", p=P))
                    nc.gpsimd.dma_start(out=binv_bc,
                                        in_=binv[None, :].to_broadcast((P, D)))
                for j in range(_ND):
                    sq = lntmp.tile([P, NCK], bf16, tag="sq", bufs=4)
                    with nc.allow_low_precision(reason="x^2 for LN stats in bf16"):
                        nc.vector.tensor_tensor(out=sq, in0=xT_sb[:, j, sl],
                                                in1=xT_sb[:, j, sl], op=OP.mult)
                    nc.tensor.matmul(sx_ps[:, sl], lhsT=ones_mat, rhs=xT_sb[:, j, sl],
                                     start=(j == 0), stop=(j == _ND - 1))
                    nc.tensor.matmul(sx2_ps[:, sl], lhsT=ones_mat, rhs=sq,
                                     start=(j == 0), stop=(j == _ND - 1))

                with nc.allow_low_precision(reason="LN stats chain in bf16"):
                    mu_bc = lnrow.tile([P, NCK], bf16, tag="mu", bufs=2)
                    nc.vector.tensor_scalar_mul(mu_bc, sx_ps[:, sl], 1.0 / D)
                    var_bc = lnrow.tile([P, NCK], f32, tag="var", bufs=2)
                    nc.vector.tensor_scalar_mul(var_bc, sx2_ps[:, sl], 1.0 / D)
                    musq = lnrow.tile([P, NCK], bf16, tag="musq", bufs=2)
                    nc.vector.tensor_tensor(out=musq, in0=mu_bc, in1=mu_bc, op=OP.mult)
                    nc.vector.tensor_tensor(out=var_bc, in0=var_bc, in1=musq,
                                            op=OP.subtract)
                    std_bc = lnrow.tile([P, NCK], bf16, tag="std", bufs=2)
                    nc.scalar.activation(out=std_bc, in_=var_bc, func=FT.Sqrt,
                                         bias=eps_sb)
                    b_bc = lnrow.tile([P, NCK], bf16, tag="b", bufs=2)
                    nc.vector.reciprocal(out=b_bc, in_=std_bc)
                    mub_bc = std_bc
                    nc.vector.tensor_tensor(out=mub_bc, in0=mu_bc, in1=b_bc,
                                            op=OP.mult)

                    for j in range(_ND):
                        t = lntmp.tile([P, NCK], bf16, tag="nrm", bufs=4)
                        eng = nc.vector if j < 5 else nc.gpsimd
                        eng.tensor_tensor(out=t, in0=xT_sb[:, j, sl],
                                          in1=b_bc, op=OP.mult)
                        eng.tensor_tensor(out=xn8[:, j, sl], in0=t,
                                          in1=mub_bc, op=OP.subtract)

        # ============ Phases 2-4: projections + attention + out-proj ========
        with tc.tile_pool(name="expool", bufs=1) as expool, \
             tc.tile_pool(name="sidep", bufs=1) as sidep, \
             tc.tile_pool(name="mps", bufs=1, space="PSUM") as mps:

            def dr_matmul(ps_out, lhsT, rhs, start, stop):
                nc.tensor.matmul(ps_out, lhsT=lhsT, rhs=rhs, start=start,
                                 stop=stop, perf_mode=DR)

            def veng(i):
                return nc.vector if i % 2 == 0 else nc.gpsimd

            def emit_qk_half(et, half, on_act=False):
                # e-tile et (128 cols of q|k), s-half: one [128, 512] group
                ps = mps.tile([P, NCK], f32, tag="mm", bufs=2,
                              name=f"qk{et}_{half}")
                e0 = et * P
                for jp in range(_ND // 2):
                    for c2 in range(2):
                        sl = slice(half * NCK + c2 * 256,
                                   half * NCK + (c2 + 1) * 256)
                        dr_matmul(
                            ps[:, c2 * 256:(c2 + 1) * 256],
                            wqk_sb[:, 2 * jp:2 * jp + 2, e0:e0 + P],
                            xn8[:, 2 * jp:2 * jp + 2, sl],
                            start=(jp == 0 and c2 == 0),
                            stop=(jp == _ND // 2 - 1 and c2 == 1))
                sl = slice(half * NCK, (half + 1) * NCK)
                with nc.allow_low_precision(reason="qk to bf16"):
                    if on_act:
                        nc.scalar.activation(out=qkT[:, et, sl], in_=ps,
                                             func=FT.Identity,
                                             bias=binqk_sb[:, et:et + 1])
                    else:
                        nc.vector.tensor_scalar_add(qkT[:, et, sl], ps,
                                                    binqk_sb[:, et:et + 1])

            def emit_v_unit(st):
                # t-tile st: V natural [128 t, 512 e'] per e'-half
                for eh in range(2):
                    ps = mps.tile([P, NCK], f32, tag="mm", bufs=2,
                                  name=f"v{st}_{eh}")
                    t0 = st * P
                    for jp in range(_ND // 2):
                        for c2 in range(2):
                            sl = slice(eh * NCK + c2 * 256,
                                       eh * NCK + (c2 + 1) * 256)
                            dr_matmul(
                                ps[:, c2 * 256:(c2 + 1) * 256],
                                xn8[:, 2 * jp:2 * jp + 2, t0:t0 + P],
                                wv_sb[:, 2 * jp:2 * jp + 2, sl],
                                start=(jp == 0 and c2 == 0),
                                stop=(jp == _ND // 2 - 1 and c2 == 1))
                    with nc.allow_low_precision(reason="v to fp8"):
                        nc.vector.tensor_tensor(
                            out=v8[:, st, eh * 8:(eh + 1) * 8, :],
                            in0=ps.rearrange("p (h d) -> p h d", d=DH),
                            in1=binv_bc[:, eh * NCK:(eh + 1) * NCK]
                                .rearrange("p (h d) -> p h d", d=DH),
                            op=OP.add)

            def emit_scores(hp):
                # per head pair: scores^T into [128, 1024] psum tiles, then a
                # single wide exp (fp8 out) per (tt, idx)
                ex_t = expool.tile([P, 2, NS, S], fp8, tag="ex", bufs=2,
                                   name=f"ex{hp}")
                for tt in range(NS):
                    for idx in range(2):
                        base = idx * DH
                        ps = mps.tile([P, S], f32, tag="sc", bufs=2,
                                      name=f"sc{hp}_{tt}_{idx}")
                        for sh in range(2):
                            sl = slice(sh * NCK, (sh + 1) * NCK)
                            nc.tensor.matmul(
                                ps[:, sl],
                                lhsT=qkT[base:base + DH, 8 + hp, tt * P:(tt + 1) * P],
                                rhs=qkT[base:base + DH, hp, sl],
                                start=True, stop=True, tile_position=(base, 0))
                        with nc.allow_low_precision(reason="exp to fp8"):
                            nc.scalar.activation(out=ex_t[:, idx, tt, :],
                                                 in_=ps, func=FT.Exp,
                                                 scale=EXP_SCALE, bias=expb)
                return ex_t

            def emit_pvden(hp, ex_t):
                # PV + denominator (DoubleRow, planes = t-tile pairs), then
                # normalize ctx^T in fp8.  den rides the "mm" psum tag and
                # arrives replicated over 64 partitions (no broadcast needed).
                for sh in range(2):
                    for idx in range(2):
                        h = 2 * hp + idx
                        ctxps = mps.tile([DH, NCK], f32, tag="ctx", bufs=2,
                                         name=f"ctx{hp}_{sh}_{idx}")
                        denft = mps.tile([P, NCK], f32, tag="mm", bufs=2,
                                         name=f"den{hp}_{sh}_{idx}")
                        denps = denft[0:DH, :]
                        for ttp in range(NS // 2):
                            for c2 in range(2):
                                sl = slice(sh * NCK + c2 * 256,
                                           sh * NCK + (c2 + 1) * 256)
                                co = slice(c2 * 256, (c2 + 1) * 256)
                                st_ = (ttp == 0 and c2 == 0)
                                sp_ = (ttp == NS // 2 - 1 and c2 == 1)
                                dr_matmul(ctxps[:, co],
                                          v8[:, 2 * ttp:2 * ttp + 2, h, :],
                                          ex_t[:, idx, 2 * ttp:2 * ttp + 2, sl],
                                          start=st_, stop=sp_)
                                dr_matmul(denps[:, co], ones8_pl,
                                          ex_t[:, idx, 2 * ttp:2 * ttp + 2, sl],
                                          start=st_, stop=sp_)
                        sl = slice(sh * NCK, (sh + 1) * NCK)
                        rden = sidep.tile([DH, NCK], bf16, tag="rd", bufs=4,
                                          name=f"rd{hp}_{sh}_{idx}")
                        with nc.allow_low_precision(reason="denom in bf16"):
                            nc.vector.reciprocal(out=rden, in_=denps)
                            nc.vector.tensor_tensor(
                                out=ctxT8[idx * DH:(idx + 1) * DH, hp, sl],
                                in0=ctxps, in1=rden, op=OP.mult)

            def emit_outproj():
                for st in range(NS):
                    for eh in range(2):
                        ps = mps.tile([P, NCK], f32, tag="mm", bufs=2,
                                      name=f"op{st}_{eh}")
                        s0 = st * P
                        for hpp in range(_ND // 2):
                            for c2 in range(2):
                                sl = slice(eh * NCK + c2 * 256,
                                           eh * NCK + (c2 + 1) * 256)
                                dr_matmul(
                                    ps[:, c2 * 256:(c2 + 1) * 256],
                                    ctxT8[:, 2 * hpp:2 * hpp + 2, s0:s0 + P],
                                    wout_sb[:, 2 * hpp:2 * hpp + 2, sl],
                                    start=(hpp == 0 and c2 == 0),
                                    stop=(hpp == _ND // 2 - 1 and c2 == 1))
                        sl = slice(eh * NCK, (eh + 1) * NCK)
                        ot = sidep.tile([P, NCK], bf16, tag="ot", bufs=4,
                                        name=f"ot{st}_{eh}")
                        ob = sidep.tile([P, NCK], bf16, tag="ob", bufs=4,
                                        name=f"ob{st}_{eh}")
                        with nc.allow_low_precision(reason="out in bf16"):
                            nc.scalar.mul(ot, ps, OUT_SCALE)
                            nc.vector.tensor_tensor(out=ob, in0=ot,
                                                    in1=resid_sb[:, st, sl],
                                                    op=OP.add)
                        nc.sync.dma_start(out=out[st * P:(st + 1) * P, sl], in_=ob)

            # ---- interleaved emission ----
            # s-half-0 work first: runs while LayerNorm's second chunk is
            # still on DVE, keeping the PE busy
            for et in range(2 * _ND):
                emit_qk_half(et, 0, on_act=True)
            for st in range(NS // 2):
                emit_v_unit(st)
            # pair 0/1 second halves, then steady-state interleave
            emit_qk_half(0, 1, on_act=True)
            emit_qk_half(8, 1, on_act=True)
            ex_prev = emit_scores(0)
            emit_qk_half(1, 1, on_act=True)
            emit_qk_half(9, 1, on_act=True)
            ex_cur = emit_scores(1)
            for st in range(NS // 2, NS):
                emit_v_unit(st)
            emit_pvden(0, ex_prev)
            ex_prev = ex_cur
            for p in range(2, H // 2):
                emit_qk_half(p, 1)
                emit_qk_half(8 + p, 1)
                ex_cur = emit_scores(p)
                emit_pvden(p - 1, ex_prev)
                ex_prev = ex_cur
            emit_pvden(H // 2 - 1, ex_prev)
            emit_outproj()


def build_nc():
    import concourse.bacc as bacc
    import concourse.tile as tile
    from concourse import mybir

    f32 = mybir.dt.float32
    bf16 = mybir.dt.bfloat16
    fp8 = mybir.dt.float8e4

    nc = bacc.Bacc("TRN2", target_bir_lowering=False, debug=False)
    aps = {
        "xt": nc.dram_tensor("xt", [D, S], bf16, kind="ExternalInput").ap(),
        "resid": nc.dram_tensor("resid", [S, D], bf16, kind="ExternalInput").ap(),
        "wqkt": nc.dram_tensor("wqkt", [D, 2 * D], fp8, kind="ExternalInput").ap(),
        "wvt": nc.dram_tensor("wvt", [D, D], fp8, kind="ExternalInput").ap(),
        "woutt": nc.dram_tensor("woutt", [D, D], fp8, kind="ExternalInput").ap(),
        "binqk": nc.dram_tensor("binqk", [P, H], f32, kind="ExternalInput").ap(),
        "binv": nc.dram_tensor("binv", [D], f32, kind="ExternalInput").ap(),
        "out": nc.dram_tensor("out", [S, D], bf16, kind="ExternalOutput").ap(),
    }
    with tile.TileContext(nc) as tc:
        _emit(tc, aps)
    nc.compile()
    return nc


def prep_inputs(x, ln_gamma, ln_beta, in_proj_w, in_proj_b, out_proj_w, out_proj_b,
                n_cores=N_CORES):
    bf = ml_dtypes.bfloat16
    f8 = ml_dtypes.float8_e4m3
    f32c = lambda a: np.ascontiguousarray(a, dtype=np.float32)
    win = np.asarray(in_proj_w, np.float32)
    g = np.asarray(ln_gamma, np.float32)
    bt = np.asarray(ln_beta, np.float32)
    bin_ = np.asarray(in_proj_b, np.float32)
    wing = win * g[None, :]          # gamma folded into in-proj columns
    binf = bin_ + win @ bt           # beta folded into the in-proj biases
    shared = {
        "wqkt": np.ascontiguousarray((wing[:2 * D] * WS).T).astype(f8),
        "wvt": np.ascontiguousarray((wing[2 * D:] * WS).T).astype(f8),
        "woutt": np.ascontiguousarray(np.asarray(out_proj_w, np.float32).T * WS).astype(f8),
        "binqk": f32c((binf[:2 * D] * WS).reshape(H, P).T),
        "binv": f32c(binf[2 * D:] * WS),
    }
    bout = np.asarray(out_proj_b, np.float32)
    in_maps = []
    for i in range(n_cores):
        xi = np.asarray(x[i], np.float32)
        m = dict(shared)
        m["xt"] = np.ascontiguousarray(xi.T).astype(bf)
        m["resid"] = np.ascontiguousarray(xi + bout).astype(bf)
        in_maps.append(m)
    return in_maps


def kernel(x, ln_gamma, ln_beta, in_proj_w, in_proj_b, out_proj_w, out_proj_b):
    global LAST_RESULTS
    from concourse import bass_utils

    if "nc" not in _NC_CACHE:
        _NC_CACHE["nc"] = build_nc()
    nc = _NC_CACHE["nc"]

    in_maps = prep_inputs(x, ln_gamma, ln_beta, in_proj_w, in_proj_b,
                          out_proj_w, out_proj_b)
    res = bass_utils.run_bass_kernel_spmd(nc, in_maps, core_ids=list(range(N_CORES)))
    LAST_RESULTS = res
    out = np.stack([r["out"] for r in res.results], axis=0)
    return np.ascontiguousarray(out, dtype=np.float32)


# revision 13
# speedup vs baseline: 1.3858x; 1.0594x over previous
"""Self-contained Trainium2 Bass kernel: pre-LN multi-head attention block.

Computes, for x [B=8, S=1024, D=1024] (fp32) and packed attention weights:
    out = x + out_proj(MHA(LayerNorm(x)))
matching torch nn.MultiheadAttention's explicit (non-flash) path with 16 heads.

Sharding: data-parallel over batch - core i handles batch element i; no
collectives, outputs are concatenated on the host.

Per-core strategy (fp8 DoubleRow matmuls at 2x PE throughput):
  - LN runs on transposed activations (d on partitions); stats are matmuls
    against an all-ones stationary so the sums land partition-replicated in
    PSUM; the normalize chain runs in bf16 on DVE and the gamma/beta apply
    runs on the Pool engine (tensor_scalar), writing xn directly in fp8.
  - QKV / V / PV / out-proj all run as fp8e4 DoubleRow matmuls with full
    128-wide stationaries: weights are pre-scaled by 32 on the host (power
    of two; folded back out via the softmax exp scale and the final output
    scale); each instruction contracts 2x128 d-coords at 0.5 cycles per
    output column.
  - scores^T[t,s] = K^T.T @ Q^T per head stay bf16 (K=64 contraction gains
    nothing from DoubleRow); exp runs on the scalar engine over [128, 1024]
    PSUM tiles (amortizing the fixed ACT access latency) with scale 1/8192
    and a -3 offset (cancels in softmax; keeps fp8 exp in range).
  - the softmax denominator comes from a DoubleRow matmul against an fp8
    all-ones stationary - its [64, N] output is the denominator replicated
    across 64 partitions, so the per-head normalize needs no broadcast.
  - PE emission: Q/K/V units are split by s-half so the first halves (plus
    warmup matmuls) keep the PE busy while LayerNorm finishes the second
    x chunk; per head pair the stream is qk(p+1) | scores(p) | pv+den(p-1)
    so the scalar engine's exp stream overlaps PE work throughout.
  - PSUM->SBUF copies alternate between DVE and the otherwise-idle Pool
    engine; residual + out_proj bias are pre-added on the host (bf16) and
    merged with one fused scalar_tensor_tensor: (psum * 2^-10) + resid.
"""

import numpy as np
import ml_dtypes

P = 128
D = 1024
H = 16
DH = 64
B = 8
S = 1024
LN_EPS = 1e-5
N_CORES = 8

_ND = D // P   # d tiles (8)
NS = S // P    # s tiles (8)
NCK = 512      # LN chunk / matmul moving width
WS = 32.0      # fp8 weight pre-scale (power of two)
EXP_SCALE = 0.125 / (WS * WS)   # 1/8192: folds 1/sqrt(dh) and the q/k scales
EXP_BIAS = -3.0                 # cancels in softmax; keeps fp8 exp in range
OUT_SCALE = 1.0 / (WS * WS)     # folds the v/out-proj weight scales back out

LAST_RESULTS = None
_NC_CACHE = {}


def _emit(tc, aps):
    from concourse import mybir

    nc = tc.nc
    f32 = mybir.dt.float32
    bf16 = mybir.dt.bfloat16
    fp8 = mybir.dt.float8e4
    FT = mybir.ActivationFunctionType
    OP = mybir.AluOpType
    DR = mybir.MatmulPerfMode.DoubleRow

    xT, resid, wqkt, wvt, woutt, binqk, binv, out = (
        aps["xt"], aps["resid"], aps["wqkt"], aps["wvt"], aps["woutt"],
        aps["binqk"], aps["binv"], aps["out"],
    )

    with tc.tile_pool(name="consts", bufs=1) as consts, \
         tc.tile_pool(name="acts", bufs=1) as acts, \
         tc.tile_pool(name="wpool", bufs=1) as wpool:

        # ---------- constants (DMAs issued after the first x chunk) ----------
        cvec = consts.tile([P, H + 1], f32)
        binqk_sb = cvec[:, 0:H]
        eps_sb = cvec[:, H:H + 1]
        nc.vector.memset(eps_sb, LN_EPS)
        ones_mat = consts.tile([P, P], bf16)
        nc.vector.memset(ones_mat, 1.0)
        ones8 = consts.tile([P, P], fp8)
        nc.vector.memset(ones8, 1.0)
        ones8_pl = ones8.rearrange("p (a m) -> p a m", a=2)  # [P, 2, 64]
        expb = consts.tile([P, 1], f32)
        nc.vector.memset(expb, EXP_BIAS)
        binv_bc = consts.tile([P, D], f32)

        # ---------- persistent activations ----------
        xn8 = acts.tile([P, _ND, S], fp8)        # normalized x, transposed
        qkT = acts.tile([P, 2 * _ND, S], bf16)   # q tiles 0..7, k tiles 8..15
        v8 = acts.tile([P, NS, H, DH], fp8)      # v natural [t, h, dh]
        ctxT8 = acts.tile([P, _ND, S], fp8)      # normalized ctx^T (d on part)
        resid_sb = acts.tile([P, NS, D], bf16)   # x + out_proj_b, natural

        # ---------- weights (fp8, pre-scaled by WS on host) ----------
        wqk_sb = wpool.tile([P, _ND, 2 * D], fp8)
        wv_sb = wpool.tile([P, _ND, D], fp8)
        wout_sb = wpool.tile([P, _ND, D], fp8)

        # ================= Phase 1: LayerNorm =================
        with tc.tile_pool(name="lnsb", bufs=1) as lnsb, \
             tc.tile_pool(name="lnrow", bufs=1) as lnrow, \
             tc.tile_pool(name="lntmp", bufs=2) as lntmp, \
             tc.tile_pool(name="lnps", bufs=1, space="PSUM") as lnps:
            xT_sb = lnsb.tile([P, _ND, S], bf16)
            sx_ps = lnps.tile([P, S], f32, tag="sx")
            sx2_ps = lnps.tile([P, S], f32, tag="sx2")
            # PE p-state warmup: dummy matmuls chain into the LN stats so the
            # clock is at full speed (and stays there) when real work arrives
            warm_ps = lnps.tile([P, P], f32, tag="warm")
            for _ in range(24):
                nc.tensor.matmul(warm_ps, lhsT=ones_mat, rhs=ones_mat,
                                 start=True, stop=True)
            xT_r = xT.rearrange("(a p) s -> p a s", p=P)
            for c in range(S // NCK):
                sl = slice(c * NCK, (c + 1) * NCK)
                for j in range(_ND):
                    nc.sync.dma_start(out=xT_sb[:, j, sl], in_=xT_r[:, j, sl])
                if c == 1:
                    # priority order: both x chunks, small consts, then the
                    # weights in first-use order; residual/out-proj last
                    nc.sync.dma_start(out=cvec[:, 0:H], in_=binqk)
                    nc.sync.dma_start(out=wqk_sb,
                                      in_=wqkt.rearrange("(a p) e -> p a e", p=P))
                    nc.sync.dma_start(out=wv_sb,
                                      in_=wvt.rearrange("(a p) e -> p a e", p=P))
                    nc.sync.dma_start(out=wout_sb,
                                      in_=woutt.rearrange("(a p) e -> p a e", p=P))
                    nc.sync.dma_start(out=resid_sb,
                                      in_=resid.rearrange("(st p) e -> p st e", p=P))
                    nc.gpsimd.dma_start(out=binv_bc,
                                        in_=binv[None, :].to_broadcast((P, D)))
                for j in range(_ND):
                    sq = lntmp.tile([P, NCK], bf16, tag="sq", bufs=4)
                    with nc.allow_low_precision(reason="x^2 for LN stats in bf16"):
                        nc.vector.tensor_tensor(out=sq, in0=xT_sb[:, j, sl],
                                                in1=xT_sb[:, j, sl], op=OP.mult)
                    nc.tensor.matmul(sx_ps[:, sl], lhsT=ones_mat, rhs=xT_sb[:, j, sl],
                                     start=(j == 0), stop=(j == _ND - 1))
                    nc.tensor.matmul(sx2_ps[:, sl], lhsT=ones_mat, rhs=sq,
                                     start=(j == 0), stop=(j == _ND - 1))

                with nc.allow_low_precision(reason="LN stats chain in bf16"):
                    mu_bc = lnrow.tile([P, NCK], bf16, tag="mu", bufs=2)
                    nc.vector.tensor_scalar_mul(mu_bc, sx_ps[:, sl], 1.0 / D)
                    var_bc = lnrow.tile([P, NCK], f32, tag="var", bufs=2)
                    nc.vector.tensor_scalar_mul(var_bc, sx2_ps[:, sl], 1.0 / D)
                    musq = lnrow.tile([P, NCK], bf16, tag="musq", bufs=2)
                    nc.vector.tensor_tensor(out=musq, in0=mu_bc, in1=mu_bc, op=OP.mult)
                    nc.vector.tensor_tensor(out=var_bc, in0=var_bc, in1=musq,
                                            op=OP.subtract)
                    std_bc = lnrow.tile([P, NCK], bf16, tag="std", bufs=2)
                    nc.scalar.activation(out=std_bc, in_=var_bc, func=FT.Sqrt,
                                         bias=eps_sb)
                    b_bc = lnrow.tile([P, NCK], bf16, tag="b", bufs=2)
                    nc.vector.reciprocal(out=b_bc, in_=std_bc)
                    mub_bc = std_bc
                    nc.vector.tensor_tensor(out=mub_bc, in0=mu_bc, in1=b_bc,
                                            op=OP.mult)

                    for j in range(_ND):
                        t = lntmp.tile([P, NCK], bf16, tag="nrm", bufs=4)
                        eng = nc.vector if j < 5 else nc.gpsimd
                        eng.tensor_tensor(out=t, in0=xT_sb[:, j, sl],
                                          in1=b_bc, op=OP.mult)
                        eng.tensor_tensor(out=xn8[:, j, sl], in0=t,
                                          in1=mub_bc, op=OP.subtract)

        # ============ Phases 2-4: projections + attention + out-proj ========
        with tc.tile_pool(name="expool", bufs=1) as expool, \
             tc.tile_pool(name="sidep", bufs=1) as sidep, \
             tc.tile_pool(name="mps", bufs=1, space="PSUM") as mps:

            def dr_matmul(ps_out, lhsT, rhs, start, stop):
                nc.tensor.matmul(ps_out, lhsT=lhsT, rhs=rhs, start=start,
                                 stop=stop, perf_mode=DR)

            def veng(i):
                return nc.vector if i % 2 == 0 else nc.gpsimd

            def emit_qk_half(et, half, on_act=False):
                # e-tile et (128 cols of q|k), s-half: one [128, 512] group
                ps = mps.tile([P, NCK], f32, tag="mm", bufs=2,
                              name=f"qk{et}_{half}")
                e0 = et * P
                for jp in range(_ND // 2):
                    for c2 in range(2):
                        sl = slice(half * NCK + c2 * 256,
                                   half * NCK + (c2 + 1) * 256)
                        dr_matmul(
                            ps[:, c2 * 256:(c2 + 1) * 256],
                            wqk_sb[:, 2 * jp:2 * jp + 2, e0:e0 + P],
                            xn8[:, 2 * jp:2 * jp + 2, sl],
                            start=(jp == 0 and c2 == 0),
                            stop=(jp == _ND // 2 - 1 and c2 == 1))
                sl = slice(half * NCK, (half + 1) * NCK)
                with nc.allow_low_precision(reason="qk to bf16"):
                    if on_act:
                        nc.scalar.activation(out=qkT[:, et, sl], in_=ps,
                                             func=FT.Identity,
                                             bias=binqk_sb[:, et:et + 1])
                    else:
                        nc.vector.tensor_scalar_add(qkT[:, et, sl], ps,
                                                    binqk_sb[:, et:et + 1])

            def emit_v_unit(st):
                # t-tile st: V natural [128 t, 512 e'] per e'-half
                for eh in range(2):
                    ps = mps.tile([P, NCK], f32, tag="mm", bufs=2,
                                  name=f"v{st}_{eh}")
                    t0 = st * P
                    for jp in range(_ND // 2):
                        for c2 in range(2):
                            sl = slice(eh * NCK + c2 * 256,
                                       eh * NCK + (c2 + 1) * 256)
                            dr_matmul(
                                ps[:, c2 * 256:(c2 + 1) * 256],
                                xn8[:, 2 * jp:2 * jp + 2, t0:t0 + P],
                                wv_sb[:, 2 * jp:2 * jp + 2, sl],
                                start=(jp == 0 and c2 == 0),
                                stop=(jp == _ND // 2 - 1 and c2 == 1))
                    with nc.allow_low_precision(reason="v to fp8"):
                        nc.vector.tensor_tensor(
                            out=v8[:, st, eh * 8:(eh + 1) * 8, :],
                            in0=ps.rearrange("p (h d) -> p h d", d=DH),
                            in1=binv_bc[:, eh * NCK:(eh + 1) * NCK]
                                .rearrange("p (h d) -> p h d", d=DH),
                            op=OP.add)

            def emit_scores(hp):
                # per head pair: scores^T into [128, 1024] psum tiles, then a
                # single wide exp (fp8 out) per (tt, idx)
                ex_t = expool.tile([P, 2, NS, S], fp8, tag="ex", bufs=2,
                                   name=f"ex{hp}")
                for tt in range(NS):
                    for idx in range(2):
                        base = idx * DH
                        ps = mps.tile([P, S], f32, tag="sc", bufs=2,
                                      name=f"sc{hp}_{tt}_{idx}")
                        for sh in range(2):
                            sl = slice(sh * NCK, (sh + 1) * NCK)
                            nc.tensor.matmul(
                                ps[:, sl],
                                lhsT=qkT[base:base + DH, 8 + hp, tt * P:(tt + 1) * P],
                                rhs=qkT[base:base + DH, hp, sl],
                                start=True, stop=True, tile_position=(base, 0))
                        with nc.allow_low_precision(reason="exp to fp8"):
                            nc.scalar.activation(out=ex_t[:, idx, tt, :],
                                                 in_=ps, func=FT.Exp,
                                                 scale=EXP_SCALE, bias=expb)
                return ex_t

            def emit_pvden(hp, ex_t):
                # PV + denominator (DoubleRow, planes = t-tile pairs), then
                # normalize ctx^T in fp8.  den rides the "mm" psum tag and
                # arrives replicated over 64 partitions (no broadcast needed).
                for sh in range(2):
                    for idx in range(2):
                        h = 2 * hp + idx
                        ctxps = mps.tile([DH, NCK], f32, tag="ctx", bufs=2,
                                         name=f"ctx{hp}_{sh}_{idx}")
                        denft = mps.tile([P, NCK], f32, tag="mm", bufs=2,
                                         name=f"den{hp}_{sh}_{idx}")
                        denps = denft[0:DH, :]
                        for ttp in range(NS // 2):
                            for c2 in range(2):
                                sl = slice(sh * NCK + c2 * 256,
                                           sh * NCK + (c2 + 1) * 256)
                                co = slice(c2 * 256, (c2 + 1) * 256)
                                st_ = (ttp == 0 and c2 == 0)
                                sp_ = (ttp == NS // 2 - 1 and c2 == 1)
                                dr_matmul(ctxps[:, co],
                                          v8[:, 2 * ttp:2 * ttp + 2, h, :],
                                          ex_t[:, idx, 2 * ttp:2 * ttp + 2, sl],
                                          start=st_, stop=sp_)
                                dr_matmul(denps[:, co], ones8_pl,
                                          ex_t[:, idx, 2 * ttp:2 * ttp + 2, sl],
                                          start=st_, stop=sp_)
                        sl = slice(sh * NCK, (sh + 1) * NCK)
                        rden = sidep.tile([DH, NCK], bf16, tag="rd", bufs=4,
                                          name=f"rd{hp}_{sh}_{idx}")
                        with nc.allow_low_precision(reason="denom in bf16"):
                            nc.vector.reciprocal(out=rden, in_=denps)
                            nc.vector.tensor_tensor(
                                out=ctxT8[idx * DH:(idx + 1) * DH, hp, sl],
                                in0=ctxps, in1=rden, op=OP.mult)

            def emit_outproj():
                for st in range(NS):
                    for eh in range(2):
                        ps = mps.tile([P, NCK], f32, tag="mm", bufs=2,
                                      name=f"op{st}_{eh}")
                        s0 = st * P
                        for hpp in range(_ND // 2):
                            for c2 in range(2):
                                sl = slice(eh * NCK + c2 * 256,
                                           eh * NCK + (c2 + 1) * 256)
                                dr_matmul(
                                    ps[:, c2 * 256:(c2 + 1) * 256],
                                    ctxT8[:, 2 * hpp:2 * hpp + 2, s0:s0 + P],
                                    wout_sb[:, 2 * hpp:2 * hpp + 2, sl],
                                    start=(hpp == 0 and c2 == 0),
                                    stop=(hpp == _ND // 2 - 1 and c2 == 1))
                        sl = slice(eh * NCK, (eh + 1) * NCK)
                        ot = sidep.tile([P, NCK], bf16, tag="ot", bufs=4,
                                        name=f"ot{st}_{eh}")
                        ob = sidep.tile([P, NCK], bf16, tag="ob", bufs=4,
                                        name=f"ob{st}_{eh}")
                        with nc.allow_low_precision(reason="out in bf16"):
                            nc.scalar.mul(ot, ps, OUT_SCALE)
                            nc.vector.tensor_tensor(out=ob, in0=ot,
                                                    in1=resid_sb[:, st, sl],
                                                    op=OP.add)
                        nc.sync.dma_start(out=out[st * P:(st + 1) * P, sl], in_=ob)

            # ---- interleaved emission ----
            # s-half-0 work first: runs while LayerNorm's second chunk is
            # still on DVE, keeping the PE busy
            for et in range(2 * _ND):
                emit_qk_half(et, 0, on_act=True)
            # pair 0/1 second halves, then steady-state interleave
            emit_qk_half(0, 1, on_act=True)
            emit_qk_half(8, 1, on_act=True)
            ex_prev = emit_scores(0)
            emit_qk_half(1, 1, on_act=True)
            emit_qk_half(9, 1, on_act=True)
            ex_cur = emit_scores(1)
            for st in range(NS):
                emit_v_unit(st)
            emit_pvden(0, ex_prev)
            ex_prev = ex_cur
            for p in range(2, H // 2):
                emit_qk_half(p, 1)
                emit_qk_half(8 + p, 1)
                ex_cur = emit_scores(p)
                emit_pvden(p - 1, ex_prev)
                ex_prev = ex_cur
            emit_pvden(H // 2 - 1, ex_prev)
            emit_outproj()


def build_nc():
    import concourse.bacc as bacc
    import concourse.tile as tile
    from concourse import mybir

    f32 = mybir.dt.float32
    bf16 = mybir.dt.bfloat16
    fp8 = mybir.dt.float8e4

    nc = bacc.Bacc("TRN2", target_bir_lowering=False, debug=False)
    aps = {
        "xt": nc.dram_tensor("xt", [D, S], bf16, kind="ExternalInput").ap(),
        "resid": nc.dram_tensor("resid", [S, D], bf16, kind="ExternalInput").ap(),
        "wqkt": nc.dram_tensor("wqkt", [D, 2 * D], fp8, kind="ExternalInput").ap(),
        "wvt": nc.dram_tensor("wvt", [D, D], fp8, kind="ExternalInput").ap(),
        "woutt": nc.dram_tensor("woutt", [D, D], fp8, kind="ExternalInput").ap(),
        "binqk": nc.dram_tensor("binqk", [P, H], f32, kind="ExternalInput").ap(),
        "binv": nc.dram_tensor("binv", [D], f32, kind="ExternalInput").ap(),
        "out": nc.dram_tensor("out", [S, D], bf16, kind="ExternalOutput").ap(),
    }
    with tile.TileContext(nc) as tc:
        _emit(tc, aps)
    nc.compile()
    return nc


def prep_inputs(x, ln_gamma, ln_beta, in_proj_w, in_proj_b, out_proj_w, out_proj_b,
                n_cores=N_CORES):
    bf = ml_dtypes.bfloat16
    f8 = ml_dtypes.float8_e4m3
    f32c = lambda a: np.ascontiguousarray(a, dtype=np.float32)
    win = np.asarray(in_proj_w, np.float32)
    g = np.asarray(ln_gamma, np.float32)
    bt = np.asarray(ln_beta, np.float32)
    bin_ = np.asarray(in_proj_b, np.float32)
    wing = win * g[None, :]          # gamma folded into in-proj columns
    binf = bin_ + win @ bt           # beta folded into the in-proj biases
    shared = {
        "wqkt": np.ascontiguousarray((wing[:2 * D] * WS).T).astype(f8),
        "wvt": np.ascontiguousarray((wing[2 * D:] * WS).T).astype(f8),
        "woutt": np.ascontiguousarray(np.asarray(out_proj_w, np.float32).T * WS).astype(f8),
        "binqk": f32c((binf[:2 * D] * WS).reshape(H, P).T),
        "binv": f32c(binf[2 * D:] * WS),
    }
    bout = np.asarray(out_proj_b, np.float32)
    in_maps = []
    for i in range(n_cores):
        xi = np.asarray(x[i], np.float32)
        m = dict(shared)
        m["xt"] = np.ascontiguousarray(xi.T).astype(bf)
        m["resid"] = np.ascontiguousarray(xi + bout).astype(bf)
        in_maps.append(m)
    return in_maps


def kernel(x, ln_gamma, ln_beta, in_proj_w, in_proj_b, out_proj_w, out_proj_b):
    global LAST_RESULTS
    from concourse import bass_utils

    if "nc" not in _NC_CACHE:
        _NC_CACHE["nc"] = build_nc()
    nc = _NC_CACHE["nc"]

    in_maps = prep_inputs(x, ln_gamma, ln_beta, in_proj_w, in_proj_b,
                          out_proj_w, out_proj_b)
    res = bass_utils.run_bass_kernel_spmd(nc, in_maps, core_ids=list(range(N_CORES)))
    LAST_RESULTS = res
    out = np.stack([r["out"] for r in res.results], axis=0)
    return np.ascontiguousarray(out, dtype=np.float32)


# revision 14
# speedup vs baseline: 1.4350x; 1.0355x over previous
"""Self-contained Trainium2 Bass kernel: pre-LN multi-head attention block.

Computes, for x [B=8, S=1024, D=1024] (fp32) and packed attention weights:
    out = x + out_proj(MHA(LayerNorm(x)))
matching torch nn.MultiheadAttention's explicit (non-flash) path with 16 heads.

Sharding: data-parallel over batch - core i handles batch element i; no
collectives, outputs are concatenated on the host.

Per-core strategy (fp8 DoubleRow matmuls at 2x PE throughput):
  - LN runs on transposed activations (d on partitions); stats are matmuls
    against an all-ones stationary so the sums land partition-replicated in
    PSUM; the normalize chain runs in bf16 on DVE and the gamma/beta apply
    runs on the Pool engine (tensor_scalar), writing xn directly in fp8.
  - QKV / V / PV / out-proj all run as fp8e4 DoubleRow matmuls with full
    128-wide stationaries: weights are pre-scaled by 32 on the host (power
    of two; folded back out via the softmax exp scale and the final output
    scale); each instruction contracts 2x128 d-coords at 0.5 cycles per
    output column.
  - scores^T[t,s] = K^T.T @ Q^T per head stay bf16 (K=64 contraction gains
    nothing from DoubleRow); exp runs on the scalar engine over [128, 1024]
    PSUM tiles (amortizing the fixed ACT access latency) with scale 1/8192
    and a -3 offset (cancels in softmax; keeps fp8 exp in range).
  - the softmax denominator comes from a DoubleRow matmul against an fp8
    all-ones stationary - its [64, N] output is the denominator replicated
    across 64 partitions, so the per-head normalize needs no broadcast.
  - PE emission: Q/K/V units are split by s-half so the first halves (plus
    warmup matmuls) keep the PE busy while LayerNorm finishes the second
    x chunk; per head pair the stream is qk(p+1) | scores(p) | pv+den(p-1)
    so the scalar engine's exp stream overlaps PE work throughout.
  - PSUM->SBUF copies alternate between DVE and the otherwise-idle Pool
    engine; residual + out_proj bias are pre-added on the host (bf16) and
    merged with one fused scalar_tensor_tensor: (psum * 2^-10) + resid.
"""

import numpy as np
import ml_dtypes

P = 128
D = 1024
H = 16
DH = 64
B = 8
S = 1024
LN_EPS = 1e-5
N_CORES = 8

_ND = D // P   # d tiles (8)
NS = S // P    # s tiles (8)
NCK = 512      # LN chunk / matmul moving width
WS = 32.0      # fp8 weight pre-scale (power of two)
EXP_SCALE = 0.125 / (WS * WS)   # 1/8192: folds 1/sqrt(dh) and the q/k scales
EXP_BIAS = -3.0                 # cancels in softmax; keeps fp8 exp in range
OUT_SCALE = 1.0 / (WS * WS)     # folds the v/out-proj weight scales back out

LAST_RESULTS = None
_NC_CACHE = {}


def _emit(tc, aps):
    from concourse import mybir

    nc = tc.nc
    f32 = mybir.dt.float32
    bf16 = mybir.dt.bfloat16
    fp8 = mybir.dt.float8e4
    FT = mybir.ActivationFunctionType
    OP = mybir.AluOpType
    DR = mybir.MatmulPerfMode.DoubleRow

    xT, resid, wqkt, wvt, woutt, binqk, binv, out = (
        aps["xt"], aps["resid"], aps["wqkt"], aps["wvt"], aps["woutt"],
        aps["binqk"], aps["binv"], aps["out"],
    )

    with tc.tile_pool(name="consts", bufs=1) as consts, \
         tc.tile_pool(name="acts", bufs=1) as acts, \
         tc.tile_pool(name="wpool", bufs=1) as wpool:

        # ---------- constants (DMAs issued after the first x chunk) ----------
        cvec = consts.tile([P, H + 1], f32)
        binqk_sb = cvec[:, 0:H]
        eps_sb = cvec[:, H:H + 1]
        nc.vector.memset(eps_sb, LN_EPS)
        ones_mat = consts.tile([P, P], bf16)
        nc.vector.memset(ones_mat, 1.0)
        ones8 = consts.tile([P, P], fp8)
        nc.vector.memset(ones8, 1.0)
        ones8_pl = ones8.rearrange("p (a m) -> p a m", a=2)  # [P, 2, 64]
        expb = consts.tile([P, 1], f32)
        nc.vector.memset(expb, EXP_BIAS)
        binv_bc = consts.tile([P, D], f32)

        # ---------- persistent activations ----------
        xn8 = acts.tile([P, _ND, S], fp8)        # normalized x, transposed
        qkT = acts.tile([P, 2 * _ND, S], bf16)   # q tiles 0..7, k tiles 8..15
        v8 = acts.tile([P, NS, H, DH], fp8)      # v natural [t, h, dh]
        ctxT8 = acts.tile([P, _ND, S], fp8)      # normalized ctx^T (d on part)
        resid_sb = acts.tile([P, NS, D], bf16)   # x + out_proj_b, natural

        # ---------- weights (fp8, pre-scaled by WS on host) ----------
        wqk_sb = wpool.tile([P, _ND, 2 * D], fp8)
        wv_sb = wpool.tile([P, _ND, D], fp8)
        wout_sb = wpool.tile([P, _ND, D], fp8)

        # ================= Phase 1: LayerNorm =================
        with tc.tile_pool(name="lnsb", bufs=1) as lnsb, \
             tc.tile_pool(name="lnrow", bufs=1) as lnrow, \
             tc.tile_pool(name="lntmp", bufs=2) as lntmp, \
             tc.tile_pool(name="lnps", bufs=1, space="PSUM") as lnps:
            xT_sb = lnsb.tile([P, _ND, S], bf16)
            sx_ps = lnps.tile([P, S], f32, tag="sx")
            sx2_ps = lnps.tile([P, S], f32, tag="sx2")
            # PE p-state warmup: dummy matmuls chain into the LN stats so the
            # clock is at full speed (and stays there) when real work arrives
            warm_ps = lnps.tile([P, P], f32, tag="warm")
            for _ in range(24):
                nc.tensor.matmul(warm_ps, lhsT=ones_mat, rhs=ones_mat,
                                 start=True, stop=True)
            xT_r = xT.rearrange("(a p) s -> p a s", p=P)
            for c in range(S // NCK):
                sl = slice(c * NCK, (c + 1) * NCK)
                for j in range(_ND):
                    nc.sync.dma_start(out=xT_sb[:, j, sl], in_=xT_r[:, j, sl])
                if c == 1:
                    # priority order: both x chunks, small consts, then the
                    # weights in first-use order; residual/out-proj last
                    nc.sync.dma_start(out=cvec[:, 0:H], in_=binqk)
                    nc.sync.dma_start(out=wqk_sb,
                                      in_=wqkt.rearrange("(a p) e -> p a e", p=P))
                    nc.sync.dma_start(out=wv_sb,
                                      in_=wvt.rearrange("(a p) e -> p a e", p=P))
                    nc.sync.dma_start(out=wout_sb,
                                      in_=woutt.rearrange("(a p) e -> p a e", p=P))
                    nc.sync.dma_start(out=resid_sb,
                                      in_=resid.rearrange("(st p) e -> p st e", p=P))
                    nc.gpsimd.dma_start(out=binv_bc,
                                        in_=binv[None, :].to_broadcast((P, D)))
                for j in range(_ND):
                    sq = lntmp.tile([P, NCK], bf16, tag="sq", bufs=4)
                    with nc.allow_low_precision(reason="x^2 for LN stats in bf16"):
                        nc.vector.tensor_tensor(out=sq, in0=xT_sb[:, j, sl],
                                                in1=xT_sb[:, j, sl], op=OP.mult)
                    nc.tensor.matmul(sx_ps[:, sl], lhsT=ones_mat, rhs=xT_sb[:, j, sl],
                                     start=(j == 0), stop=(j == _ND - 1))
                    nc.tensor.matmul(sx2_ps[:, sl], lhsT=ones_mat, rhs=sq,
                                     start=(j == 0), stop=(j == _ND - 1))

                with nc.allow_low_precision(reason="LN stats chain in bf16"):
                    mu_bc = lnrow.tile([P, NCK], bf16, tag="mu", bufs=2)
                    nc.vector.tensor_scalar_mul(mu_bc, sx_ps[:, sl], 1.0 / D)
                    var_bc = lnrow.tile([P, NCK], f32, tag="var", bufs=2)
                    nc.vector.tensor_scalar_mul(var_bc, sx2_ps[:, sl], 1.0 / D)
                    musq = lnrow.tile([P, NCK], bf16, tag="musq", bufs=2)
                    nc.vector.tensor_tensor(out=musq, in0=mu_bc, in1=mu_bc, op=OP.mult)
                    nc.vector.tensor_tensor(out=var_bc, in0=var_bc, in1=musq,
                                            op=OP.subtract)
                    std_bc = lnrow.tile([P, NCK], bf16, tag="std", bufs=2)
                    nc.scalar.activation(out=std_bc, in_=var_bc, func=FT.Sqrt,
                                         bias=eps_sb)
                    b_bc = lnrow.tile([P, NCK], bf16, tag="b", bufs=2)
                    nc.vector.reciprocal(out=b_bc, in_=std_bc)
                    mub_bc = std_bc
                    nc.vector.tensor_tensor(out=mub_bc, in0=mu_bc, in1=b_bc,
                                            op=OP.mult)

                    for j in range(_ND):
                        t = lntmp.tile([P, NCK], bf16, tag="nrm", bufs=4)
                        eng = nc.vector if j < 5 else nc.gpsimd
                        eng.tensor_tensor(out=t, in0=xT_sb[:, j, sl],
                                          in1=b_bc, op=OP.mult)
                        eng.tensor_tensor(out=xn8[:, j, sl], in0=t,
                                          in1=mub_bc, op=OP.subtract)

        # ============ Phases 2-4: projections + attention + out-proj ========
        with tc.tile_pool(name="expool", bufs=1) as expool, \
             tc.tile_pool(name="sidep", bufs=1) as sidep, \
             tc.tile_pool(name="mps", bufs=1, space="PSUM") as mps:

            def dr_matmul(ps_out, lhsT, rhs, start, stop):
                nc.tensor.matmul(ps_out, lhsT=lhsT, rhs=rhs, start=start,
                                 stop=stop, perf_mode=DR)

            def veng(i):
                return nc.vector if i % 2 == 0 else nc.gpsimd

            def emit_qk_half(et, half, on_act=False):
                # e-tile et (128 cols of q|k), s-half: one [128, 512] group
                ps = mps.tile([P, NCK], f32, tag="mm", bufs=2,
                              name=f"qk{et}_{half}")
                e0 = et * P
                for jp in range(_ND // 2):
                    for c2 in range(2):
                        sl = slice(half * NCK + c2 * 256,
                                   half * NCK + (c2 + 1) * 256)
                        dr_matmul(
                            ps[:, c2 * 256:(c2 + 1) * 256],
                            wqk_sb[:, 2 * jp:2 * jp + 2, e0:e0 + P],
                            xn8[:, 2 * jp:2 * jp + 2, sl],
                            start=(jp == 0 and c2 == 0),
                            stop=(jp == _ND // 2 - 1 and c2 == 1))
                sl = slice(half * NCK, (half + 1) * NCK)
                with nc.allow_low_precision(reason="qk to bf16"):
                    if on_act:
                        nc.scalar.activation(out=qkT[:, et, sl], in_=ps,
                                             func=FT.Identity,
                                             bias=binqk_sb[:, et:et + 1])
                    else:
                        nc.vector.tensor_scalar_add(qkT[:, et, sl], ps,
                                                    binqk_sb[:, et:et + 1])

            def emit_v_unit(st):
                # t-tile st: V natural [128 t, 512 e'] per e'-half
                for eh in range(2):
                    ps = mps.tile([P, NCK], f32, tag="mm", bufs=2,
                                  name=f"v{st}_{eh}")
                    t0 = st * P
                    for jp in range(_ND // 2):
                        for c2 in range(2):
                            sl = slice(eh * NCK + c2 * 256,
                                       eh * NCK + (c2 + 1) * 256)
                            dr_matmul(
                                ps[:, c2 * 256:(c2 + 1) * 256],
                                xn8[:, 2 * jp:2 * jp + 2, t0:t0 + P],
                                wv_sb[:, 2 * jp:2 * jp + 2, sl],
                                start=(jp == 0 and c2 == 0),
                                stop=(jp == _ND // 2 - 1 and c2 == 1))
                    with nc.allow_low_precision(reason="v to fp8"):
                        nc.vector.tensor_tensor(
                            out=v8[:, st, eh * 8:(eh + 1) * 8, :],
                            in0=ps.rearrange("p (h d) -> p h d", d=DH),
                            in1=binv_bc[:, eh * NCK:(eh + 1) * NCK]
                                .rearrange("p (h d) -> p h d", d=DH),
                            op=OP.add)

            def emit_scores(hp):
                # per head pair: scores^T into [128, 1024] psum tiles, then a
                # single wide exp (fp8 out) per (tt, idx)
                ex_t = expool.tile([P, 2, NS, S], fp8, tag="ex", bufs=2,
                                   name=f"ex{hp}")
                for tt in range(NS):
                    for idx in range(2):
                        base = idx * DH
                        ps = mps.tile([P, S], f32, tag="sc", bufs=2,
                                      name=f"sc{hp}_{tt}_{idx}")
                        for sh in range(2):
                            sl = slice(sh * NCK, (sh + 1) * NCK)
                            nc.tensor.matmul(
                                ps[:, sl],
                                lhsT=qkT[base:base + DH, 8 + hp, tt * P:(tt + 1) * P],
                                rhs=qkT[base:base + DH, hp, sl],
                                start=True, stop=True, tile_position=(base, 0))
                        with nc.allow_low_precision(reason="exp to fp8"):
                            nc.scalar.activation(out=ex_t[:, idx, tt, :],
                                                 in_=ps, func=FT.Exp,
                                                 scale=EXP_SCALE, bias=expb)
                return ex_t

            def emit_pvden(hp, ex_t):
                # PV + denominator (DoubleRow, planes = t-tile pairs), then
                # normalize ctx^T in fp8.  den rides the "mm" psum tag and
                # arrives replicated over 64 partitions (no broadcast needed).
                for sh in range(2):
                    for idx in range(2):
                        h = 2 * hp + idx
                        ctxps = mps.tile([DH, NCK], f32, tag="ctx", bufs=2,
                                         name=f"ctx{hp}_{sh}_{idx}")
                        denft = mps.tile([P, NCK], f32, tag="mm", bufs=2,
                                         name=f"den{hp}_{sh}_{idx}")
                        denps = denft[0:DH, :]
                        for ttp in range(NS // 2):
                            for c2 in range(2):
                                sl = slice(sh * NCK + c2 * 256,
                                           sh * NCK + (c2 + 1) * 256)
                                co = slice(c2 * 256, (c2 + 1) * 256)
                                st_ = (ttp == 0 and c2 == 0)
                                sp_ = (ttp == NS // 2 - 1 and c2 == 1)
                                dr_matmul(ctxps[:, co],
                                          v8[:, 2 * ttp:2 * ttp + 2, h, :],
                                          ex_t[:, idx, 2 * ttp:2 * ttp + 2, sl],
                                          start=st_, stop=sp_)
                                dr_matmul(denps[:, co], ones8_pl,
                                          ex_t[:, idx, 2 * ttp:2 * ttp + 2, sl],
                                          start=st_, stop=sp_)
                        sl = slice(sh * NCK, (sh + 1) * NCK)
                        rden = sidep.tile([DH, NCK], bf16, tag="rd", bufs=4,
                                          name=f"rd{hp}_{sh}_{idx}")
                        with nc.allow_low_precision(reason="denom in bf16"):
                            nc.vector.reciprocal(out=rden, in_=denps)
                            nc.vector.tensor_tensor(
                                out=ctxT8[idx * DH:(idx + 1) * DH, hp, sl],
                                in0=ctxps, in1=rden, op=OP.mult)

            def emit_outproj():
                for st in range(NS):
                    ps = mps.tile([P, S], f32, tag="sc", bufs=2,
                                  name=f"op{st}")
                    s0 = st * P
                    for eh in range(2):
                        for hpp in range(_ND // 2):
                            for c2 in range(2):
                                sl = slice(eh * NCK + c2 * 256,
                                           eh * NCK + (c2 + 1) * 256)
                                dr_matmul(
                                    ps[:, sl],
                                    ctxT8[:, 2 * hpp:2 * hpp + 2, s0:s0 + P],
                                    wout_sb[:, 2 * hpp:2 * hpp + 2, sl],
                                    start=(hpp == 0 and c2 == 0),
                                    stop=(hpp == _ND // 2 - 1 and c2 == 1))
                    ot = sidep.tile([P, S], bf16, tag="ot", bufs=2,
                                    name=f"ot{st}")
                    ob = sidep.tile([P, S], bf16, tag="ob", bufs=2,
                                    name=f"ob{st}")
                    with nc.allow_low_precision(reason="out in bf16"):
                        nc.scalar.mul(ot, ps, OUT_SCALE)
                        nc.vector.tensor_tensor(out=ob, in0=ot,
                                                in1=resid_sb[:, st, :],
                                                op=OP.add)
                    nc.sync.dma_start(out=out[st * P:(st + 1) * P, :], in_=ob)

            # ---- interleaved emission ----
            # s-half-0 work first: runs while LayerNorm's second chunk is
            # still on DVE, keeping the PE busy
            # pair-0 q/k first (s-half-0 while LN's second chunk runs),
            # with v fills riding the xn-c1 wait
            emit_qk_half(0, 0, on_act=True)
            emit_qk_half(8, 0, on_act=True)
            for st in range(NS // 2):
                emit_v_unit(st)
            emit_qk_half(0, 1, on_act=True)
            emit_qk_half(8, 1, on_act=True)
            ex_prev = emit_scores(0)
            for half in range(2):
                emit_qk_half(1, half)
                emit_qk_half(9, half)
            ex_cur = emit_scores(1)
            for st in range(NS // 2, NS):
                emit_v_unit(st)
            emit_pvden(0, ex_prev)
            ex_prev = ex_cur
            for p in range(2, H // 2):
                for half in range(2):
                    emit_qk_half(p, half)
                    emit_qk_half(8 + p, half)
                ex_cur = emit_scores(p)
                emit_pvden(p - 1, ex_prev)
                ex_prev = ex_cur
            emit_pvden(H // 2 - 1, ex_prev)
            emit_outproj()


def build_nc():
    import concourse.bacc as bacc
    import concourse.tile as tile
    from concourse import mybir

    f32 = mybir.dt.float32
    bf16 = mybir.dt.bfloat16
    fp8 = mybir.dt.float8e4

    nc = bacc.Bacc("TRN2", target_bir_lowering=False, debug=False)
    aps = {
        "xt": nc.dram_tensor("xt", [D, S], bf16, kind="ExternalInput").ap(),
        "resid": nc.dram_tensor("resid", [S, D], bf16, kind="ExternalInput").ap(),
        "wqkt": nc.dram_tensor("wqkt", [D, 2 * D], fp8, kind="ExternalInput").ap(),
        "wvt": nc.dram_tensor("wvt", [D, D], fp8, kind="ExternalInput").ap(),
        "woutt": nc.dram_tensor("woutt", [D, D], fp8, kind="ExternalInput").ap(),
        "binqk": nc.dram_tensor("binqk", [P, H], f32, kind="ExternalInput").ap(),
        "binv": nc.dram_tensor("binv", [D], f32, kind="ExternalInput").ap(),
        "out": nc.dram_tensor("out", [S, D], bf16, kind="ExternalOutput").ap(),
    }
    with tile.TileContext(nc) as tc:
        _emit(tc, aps)
    nc.compile()
    return nc


def prep_inputs(x, ln_gamma, ln_beta, in_proj_w, in_proj_b, out_proj_w, out_proj_b,
                n_cores=N_CORES):
    bf = ml_dtypes.bfloat16
    f8 = ml_dtypes.float8_e4m3
    f32c = lambda a: np.ascontiguousarray(a, dtype=np.float32)
    win = np.asarray(in_proj_w, np.float32)
    g = np.asarray(ln_gamma, np.float32)
    bt = np.asarray(ln_beta, np.float32)
    bin_ = np.asarray(in_proj_b, np.float32)
    wing = win * g[None, :]          # gamma folded into in-proj columns
    binf = bin_ + win @ bt           # beta folded into the in-proj biases
    shared = {
        "wqkt": np.ascontiguousarray((wing[:2 * D] * WS).T).astype(f8),
        "wvt": np.ascontiguousarray((wing[2 * D:] * WS).T).astype(f8),
        "woutt": np.ascontiguousarray(np.asarray(out_proj_w, np.float32).T * WS).astype(f8),
        "binqk": f32c((binf[:2 * D] * WS).reshape(H, P).T),
        "binv": f32c(binf[2 * D:] * WS),
    }
    bout = np.asarray(out_proj_b, np.float32)
    in_maps = []
    for i in range(n_cores):
        xi = np.asarray(x[i], np.float32)
        m = dict(shared)
        m["xt"] = np.ascontiguousarray(xi.T).astype(bf)
        m["resid"] = np.ascontiguousarray(xi + bout).astype(bf)
        in_maps.append(m)
    return in_maps


def kernel(x, ln_gamma, ln_beta, in_proj_w, in_proj_b, out_proj_w, out_proj_b):
    global LAST_RESULTS
    from concourse import bass_utils

    if "nc" not in _NC_CACHE:
        _NC_CACHE["nc"] = build_nc()
    nc = _NC_CACHE["nc"]

    in_maps = prep_inputs(x, ln_gamma, ln_beta, in_proj_w, in_proj_b,
                          out_proj_w, out_proj_b)
    res = bass_utils.run_bass_kernel_spmd(nc, in_maps, core_ids=list(range(N_CORES)))
    LAST_RESULTS = res
    out = np.stack([r["out"] for r in res.results], axis=0)
    return np.ascontiguousarray(out, dtype=np.float32)
